# revision 2
# baseline (speedup 1.0000x reference)
"""Trainium2 Bass kernel for a complex-valued attention block — v3.

Reference computation (per batch sample, complex64):
  h = ComplexGroupNorm(x); q,k,v = 1x1 complex convs of h
  attn = (q^T k) * C^-0.5; aw = magnitude-softmax(attn)
  out = v @ aw^T; y = x + 1x1conv_p(out)

Sharding: 8 cores = 4 batches x 2 query-halves (x rolled so each core's
query block is columns [0, 1152)).

v3 (vs the two-stage v1):
  - one-pass softmax per group of SG=3 key tiles with LAGGED v-matmul
    emission: the PE queue runs group g+2's score matmuls while group g's
    ln/exp chain completes, so the tensor engine never waits on softmax.
  - all matmuls fp16 (fp8 scores fail: the peaked softmax amplifies
    quantization; measured 3e-2 rel err) except the denominator, which is
    an fp8 DoubleRow ones-matmul over te pairs (te8 errs only ~2e-3).
  - EXP_SHIFT 2^-8 keeps te8 under the fp8e4 max (peak |attn| is 10.9).
  - e^mag via a 4th ACT pass writing fp8 directly; 1/den = exp(-ln(den))
    on ACT straight from PSUM; negated vT / negated q_i copies instead of
    per-chunk negations; all softmax intermediates fp16 (lbias=1e-8 keeps
    them in range); p-bias folded into the residual on the host.
"""

import os
import ml_dtypes
import numpy as np

import concourse.bacc as bacc
import concourse.bass as bass
import concourse.tile as tile
from concourse import mybir
from concourse.bass_utils import run_bass_kernel_spmd

B, C, HH, WW = 4, 256, 48, 48
N = HH * WW            # 2304 keys
NQ = N // 2            # 1152 queries per core
CK = 384               # query-chunk width
NCK = NQ // CK         # 3 chunks
MT = N // 128          # 18 key tiles
SG = 3
NG = MT // SG          # 6 softmax groups per chunk
LAG = 2                # groups between score emission and v-matmul emission
GN_EPS = 1e-6

F32 = mybir.dt.float32
FP16 = mybir.dt.float16
FP8 = mybir.dt.float8e4
EXP_SHIFT = -5.545177444479562  # -8*ln2; cancels via the denominator
LBIAS = 1e-8

AF = mybir.ActivationFunctionType
OP = mybir.AluOpType
DRM = mybir.MatmulPerfMode.DoubleRow


def _emit(nc, tc, d):
    def pool(name, bufs, space="SBUF"):
        return tc.tile_pool(name=name, bufs=bufs, space=space)

    def mm(out, lhsT, rhs, start, stop):
        nc.tensor.matmul(out, lhsT, rhs, start=start, stop=stop,
                         skip_group_check=True)

    with pool("const", 1) as const, pool("persist", 1) as pers:

        # ---- x first: GN stats are the critical path at kernel start ----
        x16 = {p_: pers.tile([128, 2, N], FP16, tag=f"x16{p_}", name=f"x16{p_}")
               for p_ in ("r", "i")}
        for p_ in ("r", "i"):
            for t in (0, 1):
                nc.sync.dma_start(out=x16[p_][:, t, :],
                                  in_=d["x16r" if p_ == "r" else "x16i"][:, t, :])

        # ---- small consts needed by the GN chain come before weight packs ----
        def vec128(nm, src, lo):
            t = const.tile([128, 1], F32, tag=nm, name=nm)
            nc.sync.dma_start(out=t, in_=src[lo:lo + 128, :])
            return t

        selmat = const.tile([128, 16], F32, tag="selmat", name="selmat")
        nc.sync.dma_start(out=selmat, in_=d["selmat"])
        expmat = const.tile([16, 128], F32, tag="expmat", name="expmat")
        nc.sync.dma_start(out=expmat, in_=d["expmat"])
        gam = {(p_, t): vec128(f"gam{p_}{t}", d["gamma_r" if p_ == "r" else "gamma_i"], t * 128)
               for p_ in ("r", "i") for t in (0, 1)}
        bet = {(p_, t): vec128(f"bet{p_}{t}", d["beta_r" if p_ == "r" else "beta_i"], t * 128)
               for p_ in ("r", "i") for t in (0, 1)}
        qkb = {}
        for nm in ("qbr", "qbi", "nqbi", "kbr", "kbi"):
            for co in (0, 1):
                qkb[(nm, co)] = vec128(f"{nm}{co}", d[nm], co * 128)
        bvrow = {}
        for nm in ("bvr", "bvi", "nbvi"):
            t = const.tile([1, 256], FP16, tag=nm, name=nm)
            nc.sync.dma_start(out=t, in_=d[nm])
            bvrow[nm] = t
        ones8 = const.tile([128, 2, 16], FP8, tag="ones8", name="ones8")
        nc.sync.dma_start(out=ones8, in_=d["ones8"])
        ones_row = const.tile([1, 128], FP16, tag="ones_row", name="ones_row")
        nc.sync.dma_start(out=ones_row, in_=d["ones_row"])
        ebias = const.tile([128, 1], F32, tag="ebias", name="ebias")
        nc.vector.memset(ebias, EXP_SHIFT)
        lbias = const.tile([128, 1], F32, tag="lbias", name="lbias")
        nc.vector.memset(lbias, LBIAS)

        # ---- weight packs last: only needed once projections start ----
        wpk = {}
        for nm in ("wqr", "wqi", "nwqi", "wkr", "wki", "nwki",
                   "wvr", "wvi", "nwvi", "wpr", "wpi", "nwpi"):
            t = const.tile([128, 2, 256], FP16, tag=nm, name=nm)
            nc.sync.dma_start(out=t, in_=d[nm])
            wpk[nm] = t

        # ---- persistent activations ----
        h16 = {p_: pers.tile([128, 2, N], FP16, tag=f"h16{p_}", name=f"h16{p_}")
               for p_ in ("r", "i")}
        q16 = {nm: pers.tile([128, 2, NQ], FP16, tag=f"q16{nm}", name=f"q16{nm}")
               for nm in ("r", "i", "ni")}
        k16 = {nm: pers.tile([128, 2, N], FP16, tag=f"k16{nm}", name=f"k16{nm}")
               for nm in ("r", "i")}
        vT = {nm: pers.tile([128, MT, 256], FP16, tag=f"vT{nm}", name=f"vT{nm}")
              for nm in ("r", "i", "ni")}

        # ================= phase 1: GroupNorm -> h16 packs =================
        with pool("gnw", 2) as gnw, pool("ps_small", 1, "PSUM") as ps_small, \
             nc.named_scope("groupnorm"):
            mv = {}
            for p_ in ("r", "i"):
                for t in (0, 1):
                    xv = x16[p_][:, t, :].rearrange("p (s f) -> p s f", f=256)
                    st = gnw.tile([128, 9, 6], F32, tag="bnstats", name="bnstats")
                    for s9 in range(9):
                        nc.vector.bn_stats(out=st[:, s9, :], in_=xv[:, s9, :])
                    m = gnw.tile([128, 2], F32, tag=f"mv{p_}{t}", name=f"mv{p_}{t}", bufs=1)
                    nc.vector.bn_aggr(out=m, in_=st)
                    mv[(p_, t)] = m
            # merged t=0/1 aggregation: srhs [128, 8] = [t, part, (mean, E2)]
            srhs = gnw.tile([128, 8], F32, tag="srhs", name="srhs", bufs=1)
            for t in (0, 1):
                for ci, p_ in enumerate(("r", "i")):
                    m = mv[(p_, t)]
                    o = 4 * t + 2 * ci
                    nc.vector.tensor_copy(out=srhs[:, o:o + 1], in_=m[:, 0:1])
                    tmp = gnw.tile([128, 1], F32, tag="gtmp", name="gtmp")
                    nc.vector.tensor_tensor(out=tmp, in0=m[:, 0:1], in1=m[:, 0:1], op=OP.mult)
                    nc.vector.tensor_tensor(out=srhs[:, o + 1:o + 2],
                                            in0=tmp, in1=m[:, 1:2], op=OP.add)
            psg = ps_small.tile([16, 8], F32, tag="psg", name="psg")
            nc.tensor.matmul(psg, selmat, srhs, start=True, stop=True)
            gsum = gnw.tile([16, 8], F32, tag="gsum", name="gsum", bufs=1)
            nc.scalar.copy(out=gsum, in_=psg)
            gv = gsum[:].rearrange("p (t c) -> p t c", c=4)
            # st4 [16, (mu_t0, rstd_t0, mu_t1, rstd_t1)]
            st4 = gnw.tile([16, 2, 2], F32, tag="st4", name="st4", bufs=1)
            mu = st4[:, :, 0:1]
            nc.vector.tensor_tensor(out=mu, in0=gv[:, :, 0:1], in1=gv[:, :, 2:3], op=OP.add)
            nc.vector.tensor_scalar_mul(mu, mu, 1.0 / 16.0)
            ex2 = gnw.tile([16, 2], F32, tag="gex2", name="gex2")
            nc.vector.tensor_tensor(out=ex2, in0=gv[:, :, 1:2], in1=gv[:, :, 3:4], op=OP.add)
            nc.vector.tensor_scalar_mul(ex2, ex2, 1.0 / 16.0)
            mu2 = gnw.tile([16, 2], F32, tag="gmu2", name="gmu2")
            nc.vector.tensor_tensor(out=mu2, in0=mu[:, :, 0], in1=mu[:, :, 0], op=OP.mult)
            var = gnw.tile([16, 2], F32, tag="gvar", name="gvar")
            nc.vector.tensor_tensor(out=var, in0=ex2, in1=mu2, op=OP.subtract)
            epst = gnw.tile([16, 1], F32, tag="geps", name="geps", bufs=1)
            nc.vector.memset(epst, GN_EPS)
            lnv = gnw.tile([16, 2], F32, tag="glnv", name="glnv")
            nc.scalar.activation(out=lnv, in_=var, func=AF.Ln, bias=epst, scale=1.0)
            nc.scalar.activation(out=st4[:, :, 1:2], in_=lnv, func=AF.Exp, bias=0.0, scale=-0.5)
            psr = ps_small.tile([128, 4], F32, tag="psr", name="psr")
            nc.tensor.matmul(psr, expmat, st4, start=True, stop=True)
            gst = gnw.tile([128, 2, 2], F32, tag="gst", name="gst", bufs=1)
            nc.scalar.copy(out=gst, in_=psr)
            for p_ in ("r", "i"):
                for t in (0, 1):
                    sc = gnw.tile([128, 1], F32, tag=f"sc{p_}{t}", name=f"sc{p_}{t}", bufs=1)
                    nc.vector.tensor_tensor(out=sc, in0=gst[:, t, 1:2], in1=gam[(p_, t)], op=OP.mult)
                    bi = gnw.tile([128, 1], F32, tag=f"bi{p_}{t}", name=f"bi{p_}{t}", bufs=1)
                    nc.vector.tensor_tensor(out=bi, in0=gst[:, t, 0:1], in1=sc, op=OP.mult)
                    nc.vector.scalar_tensor_tensor(out=bi, in0=bi, scalar=-1.0,
                                                   in1=bet[(p_, t)], op0=OP.mult, op1=OP.add)
                    # apply on DVE (fp16 tensor_scalar runs 4x; ACT pays 2.2us/tile)
                    nc.vector.tensor_scalar(out=h16[p_][:, t, :], in0=x16[p_][:, t, :],
                                            scalar1=sc, scalar2=bi, op0=OP.mult, op1=OP.add)

        # ============ phase 2: q/k/vT projections (fp16) ============
        with pool("ps_proj", 4, "PSUM") as psp, nc.named_scope("qkv"):
            for ic in range(NCK):
                cols = slice(ic * CK, (ic + 1) * CK)
                for co in (0, 1):
                    co_sl = slice(co * 128, (co + 1) * 128)
                    ps_r = psp.tile([128, CK], F32, tag="pp", name="ps_qr")
                    ps_i = psp.tile([128, CK], F32, tag="pp", name="ps_qi")
                    for t in (0, 1):
                        mm(ps_r, wpk["wqr"][:, t, co_sl], h16["r"][:, t, cols], t == 0, False)
                        mm(ps_i, wpk["wqr"][:, t, co_sl], h16["i"][:, t, cols], t == 0, False)
                    for t in (0, 1):
                        mm(ps_r, wpk["nwqi"][:, t, co_sl], h16["i"][:, t, cols], False, t == 1)
                        mm(ps_i, wpk["wqi"][:, t, co_sl], h16["r"][:, t, cols], False, t == 1)
                    nc.scalar.activation(out=q16["r"][:, co, cols], in_=ps_r,
                                         func=AF.Identity, bias=qkb[("qbr", co)], scale=1.0)
                    nc.scalar.activation(out=q16["i"][:, co, cols], in_=ps_i,
                                         func=AF.Identity, bias=qkb[("qbi", co)], scale=1.0)
                    nc.vector.tensor_scalar(out=q16["ni"][:, co, cols], in0=ps_i,
                                            scalar1=-1.0, scalar2=qkb[("nqbi", co)],
                                            op0=OP.mult, op1=OP.add)
            for ic in range(N // CK):
                cols = slice(ic * CK, (ic + 1) * CK)
                for co in (0, 1):
                    co_sl = slice(co * 128, (co + 1) * 128)
                    ps_r = psp.tile([128, CK], F32, tag="pp", name="ps_kr")
                    ps_i = psp.tile([128, CK], F32, tag="pp", name="ps_ki")
                    for t in (0, 1):
                        mm(ps_r, wpk["wkr"][:, t, co_sl], h16["r"][:, t, cols], t == 0, False)
                        mm(ps_i, wpk["wkr"][:, t, co_sl], h16["i"][:, t, cols], t == 0, False)
                    for t in (0, 1):
                        mm(ps_r, wpk["nwki"][:, t, co_sl], h16["i"][:, t, cols], False, t == 1)
                        mm(ps_i, wpk["wki"][:, t, co_sl], h16["r"][:, t, cols], False, t == 1)
                    nc.vector.tensor_scalar_add(k16["r"][:, co, cols], ps_r, qkb[("kbr", co)])
                    nc.vector.tensor_scalar_add(k16["i"][:, co, cols], ps_i, qkb[("kbi", co)])
            # bv broadcast emitted here (not at kernel start) so the tiny
            # matmuls don't block the GN aggregation matmul in the PE queue
            bv_bc = {}
            for nm in ("bvr", "bvi", "nbvi"):
                psb = psp.tile([128, 256], F32, tag="pp", name="psb")
                nc.tensor.matmul(psb, ones_row[:], bvrow[nm][:], start=True, stop=True)
                t = const.tile([128, 256], F32, tag=nm + "_bc", name=nm + "_bc")
                nc.scalar.copy(out=t, in_=psb)
                bv_bc[nm] = t
            with nc.named_scope("vproj"):
                for j in range(MT):
                    msl = slice(j * 128, (j + 1) * 128)
                    ps_vr = psp.tile([128, 256], F32, tag="pp", name="ps_vr")
                    ps_vi = psp.tile([128, 256], F32, tag="pp", name="ps_vi")
                    for t in (0, 1):
                        mm(ps_vr, h16["r"][:, t, msl], wpk["wvr"][:, t, :], t == 0, False)
                        mm(ps_vi, h16["r"][:, t, msl], wpk["wvi"][:, t, :], t == 0, False)
                    for t in (0, 1):
                        mm(ps_vr, h16["i"][:, t, msl], wpk["nwvi"][:, t, :], False, t == 1)
                        mm(ps_vi, h16["i"][:, t, msl], wpk["wvr"][:, t, :], False, t == 1)
                    nc.vector.tensor_tensor(out=vT["r"][:, j, :], in0=ps_vr, in1=bv_bc["bvr"], op=OP.add)
                    nc.vector.tensor_tensor(out=vT["i"][:, j, :], in0=ps_vi, in1=bv_bc["bvi"], op=OP.add)
                    nc.vector.scalar_tensor_tensor(out=vT["ni"][:, j, :], in0=ps_vi, scalar=-1.0,
                                                   in1=bv_bc["nbvi"], op0=OP.mult, op1=OP.add)

        # ============ phase 3: attention (one pass, lagged v-matmul) ============
        with pool("psA", 3, "PSUM") as psA, pool("psACC", 1, "PSUM") as psACC, \
             pool("psD", 1, "PSUM") as psD, \
             pool("cpp", 2) as cpp, pool("smp", 2) as smp, pool("awp", LAG + 1) as awp, \
             pool("ckp", 2) as ckp:
            # per-chunk state, created lazily as the global group stream reaches it
            st = {}

            def chunk_state(ic):
                if ic not in st:
                    st[ic] = {
                        "te8": awp.tile([128, MT, CK], FP8, tag="te8", name="te8", bufs=2),
                        "psden": psD.tile([128, 512], F32, tag="den", name="psden"),
                        "accs": None,
                        "aw": {},
                        "den_done": 0,
                    }
                return st[ic]

            def attn_group(ic, g):
                cols = slice(ic * CK, (ic + 1) * CK)
                s = chunk_state(ic)
                cp = cpp.tile([128, SG, 2, CK], FP16, tag="cp", name="cp")
                for js in range(SG):
                    j = g * SG + js
                    msl = slice(j * 128, (j + 1) * 128)
                    ps_r = psA.tile([128, 512], F32, tag="pair", name="ps_r")
                    ps_i = psA.tile([128, 512], F32, tag="pair", name="ps_i")
                    for t in (0, 1):
                        mm(ps_r[:, 0:CK], k16["r"][:, t, msl], q16["r"][:, t, cols], t == 0, False)
                        mm(ps_i[:, 0:CK], k16["r"][:, t, msl], q16["i"][:, t, cols], t == 0, False)
                    for t in (0, 1):
                        mm(ps_r[:, 0:CK], k16["i"][:, t, msl], q16["ni"][:, t, cols], False, t == 1)
                        mm(ps_i[:, 0:CK], k16["i"][:, t, msl], q16["r"][:, t, cols], False, t == 1)
                    if js % 2 == 0:
                        nc.scalar.copy(out=cp[:, js, 0, :], in_=ps_r[:, 0:CK])
                        nc.vector.tensor_copy(out=cp[:, js, 1, :], in_=ps_i[:, 0:CK])
                    else:
                        nc.vector.tensor_copy(out=cp[:, js, 0, :], in_=ps_r[:, 0:CK])
                        nc.scalar.copy(out=cp[:, js, 1, :], in_=ps_i[:, 0:CK])
                sq = smp.tile([128, SG, 2, CK], FP16, tag="sq", name="sq")
                nc.vector.tensor_tensor(out=sq, in0=cp, in1=cp, op=OP.mult)
                m2 = smp.tile([128, SG, CK], FP16, tag="m2", name="m2")
                nc.vector.tensor_tensor(out=m2, in0=sq[:, :, 0, :], in1=sq[:, :, 1, :], op=OP.add)
                lf = smp.tile([128, SG, CK], FP16, tag="lf", name="lf")
                nc.scalar.activation(out=lf, in_=m2, func=AF.Ln, bias=lbias)
                mag = smp.tile([128, SG, CK], FP16, tag="mag", name="mag")
                nc.scalar.activation(out=mag, in_=lf, func=AF.Exp, bias=0.0, scale=0.5)
                u = smp.tile([128, SG, CK], FP16, tag="u", name="u")
                nc.vector.scalar_tensor_tensor(out=u, in0=lf, scalar=-0.5,
                                               in1=mag, op0=OP.mult, op1=OP.add)
                tt = smp.tile([128, SG, CK], FP16, tag="tt", name="tt")
                nc.scalar.activation(out=tt, in_=u, func=AF.Exp, bias=ebias)
                nc.scalar.activation(out=s["te8"][:, g * SG:(g + 1) * SG, :], in_=mag,
                                     func=AF.Exp, bias=ebias)
                aw = awp.tile([128, SG, 2, CK], FP16, tag="aw", name="aw")
                nc.vector.tensor_tensor(out=aw[:, :, 0, :], in0=tt, in1=cp[:, :, 0, :], op=OP.mult)
                nc.vector.tensor_tensor(out=aw[:, :, 1, :], in0=tt, in1=cp[:, :, 1, :], op=OP.mult)
                s["aw"][g] = aw

            def vmm_group(ic, g):
                s = chunk_state(ic)
                if s["accs"] is None:
                    s["accs"] = {(p_, co): psACC.tile([128, CK], F32, tag=f"acc{p_}{co}",
                                                      name=f"acc{p_}{co}")
                                 for p_ in ("r", "i") for co in (0, 1)}
                accs = s["accs"]
                aw = s["aw"].pop(g)
                for js in range(SG):
                    j = g * SG + js
                    st_ = (j == 0)
                    sp_ = (j == MT - 1)
                    for co in (0, 1):
                        co_sl = slice(co * 128, (co + 1) * 128)
                        nc.tensor.matmul(accs[("r", co)], vT["r"][:, j, co_sl], aw[:, js, 0, :], start=st_, stop=False, skip_group_check=True)
                        nc.tensor.matmul(accs[("i", co)], vT["r"][:, j, co_sl], aw[:, js, 1, :], start=st_, stop=False, skip_group_check=True)
                        nc.tensor.matmul(accs[("r", co)], vT["ni"][:, j, co_sl], aw[:, js, 1, :], start=False, stop=sp_, skip_group_check=True)
                        nc.tensor.matmul(accs[("i", co)], vT["i"][:, j, co_sl], aw[:, js, 0, :], start=False, stop=sp_, skip_group_check=True)
                hi = ((g + 1) * SG) // 2
                for jp in range(s["den_done"], hi):
                    nc.tensor.matmul(s["psden"][0:1, 0:CK], ones8[:, :, 0:1],
                                     s["te8"][:, 2 * jp:2 * jp + 2, :],
                                     start=(jp == 0), stop=(jp == MT // 2 - 1),
                                     perf_mode=DRM, skip_group_check=True)
                s["den_done"] = hi

            def tail(ic):
                cols = slice(ic * CK, (ic + 1) * CK)
                s = st.pop(ic)
                psden, accs = s["psden"], s["accs"]
                lden = ckp.tile([1, CK], F32, tag="lden", name="lden")
                nc.scalar.activation(out=lden, in_=psden[0:1, 0:CK], func=AF.Ln, bias=0.0)
                rec16 = ckp.tile([1, CK], FP16, tag="rec16", name="rec16")
                nc.scalar.activation(out=rec16, in_=lden, func=AF.Exp, bias=0.0, scale=-1.0)
                nc.tensor.matmul(psden[:, 0:CK], ones_row[:], rec16[:], start=True, stop=True, skip_group_check=True)
                sden = ckp.tile([128, CK], FP16, tag="sden", name="sden")
                nc.vector.tensor_copy(out=sden, in_=psden[:, 0:CK])

                outsc = {p_: ckp.tile([128, 2, CK], FP16, tag=f"osc{p_}", name=f"osc{p_}")
                         for p_ in ("r", "i")}
                for p_ in ("r", "i"):
                    for co in (0, 1):
                        nc.vector.tensor_tensor(out=outsc[p_][:, co, :], in0=accs[(p_, co)],
                                                in1=sden, op=OP.mult)

                for co in (0, 1):
                    co_sl = slice(co * 128, (co + 1) * 128)
                    ps_pr = psACC.tile([128, CK], F32, tag="accr0", name="ps_pr")
                    ps_pi = psACC.tile([128, CK], F32, tag="acci0", name="ps_pi")
                    for t in (0, 1):
                        mm(ps_pr, wpk["wpr"][:, t, co_sl], outsc["r"][:, t, :], t == 0, False)
                        mm(ps_pi, wpk["wpr"][:, t, co_sl], outsc["i"][:, t, :], t == 0, False)
                    for t in (0, 1):
                        mm(ps_pr, wpk["nwpi"][:, t, co_sl], outsc["i"][:, t, :], False, t == 1)
                        mm(ps_pi, wpk["wpi"][:, t, co_sl], outsc["r"][:, t, :], False, t == 1)
                    for p_, ps_p in (("r", ps_pr), ("i", ps_pi)):
                        # xr32/xi32 carry x + p-bias (host-folded)
                        xres = ckp.tile([128, CK], F32, tag=f"xres{p_}{co}",
                                        name=f"xres{p_}{co}", bufs=1)
                        nc.sync.dma_start(out=xres, in_=d["xr32" if p_ == "r" else "xi32"][co * 128:(co + 1) * 128, cols])
                        nc.vector.scalar_tensor_tensor(out=xres, in0=ps_p, scalar=1.0,
                                                       in1=xres, op0=OP.mult, op1=OP.add)
                        nc.sync.dma_start(out=d["outr" if p_ == "r" else "outi"][co * 128:(co + 1) * 128, cols], in_=xres)

            # global group stream: chunk ic's tail interleaves with chunk
            # ic+1's first attention groups, so the PE never idles on the
            # denominator/outsc/p-proj chain.
            TOT = NCK * NG
            for gg in range(TOT + LAG):
                ic, g = divmod(gg, NG)
                if gg < TOT:
                    with nc.named_scope(f"attn_chunk{ic}"):
                        attn_group(ic, g)
                vg = gg - LAG
                if vg >= 0:
                    vic, vgg = divmod(vg, NG)
                    with nc.named_scope(f"attn_chunk{vic}"):
                        vmm_group(vic, vgg)
                        if vgg == NG - 1:
                            tail(vic)


_CACHE = {}


def _build():
    if "nc" in _CACHE:
        return _CACHE["nc"]
    nc = bacc.Bacc("TRN2", target_bir_lowering=False, debug=False, num_devices=8)
    d = {}
    ins = {
        "x16r": ([128, 2, N], FP16), "x16i": ([128, 2, N], FP16),
        "xr32": ([C, N], F32), "xi32": ([C, N], F32),
        "qbr": ([C, 1], F32), "qbi": ([C, 1], F32), "nqbi": ([C, 1], F32),
        "kbr": ([C, 1], F32), "kbi": ([C, 1], F32),
        "bvr": ([1, C], FP16), "bvi": ([1, C], FP16), "nbvi": ([1, C], FP16),
        "gamma_r": ([C, 1], F32), "gamma_i": ([C, 1], F32),
        "beta_r": ([C, 1], F32), "beta_i": ([C, 1], F32),
        "selmat": ([128, 16], F32), "expmat": ([16, 128], F32),
        "ones8": ([128, 2, 16], FP8), "ones_row": ([1, 128], FP16),
    }
    for nm in ("wqr", "wqi", "nwqi", "wkr", "wki", "nwki",
               "wvr", "wvi", "nwvi", "wpr", "wpi", "nwpi"):
        ins[nm] = ([128, 2, 256], FP16)
    for nm, (sh, dt_) in ins.items():
        d[nm] = nc.dram_tensor(nm, list(sh), dt_, kind="ExternalInput").ap()
    for nm in ("outr", "outi"):
        d[nm] = nc.dram_tensor(nm, [C, NQ], F32, kind="ExternalOutput").ap()
    with tile.TileContext(nc) as tc:
        _emit(nc, tc, d)
    import concourse.bacc as _bacc_mod
    _orig_tables = _bacc_mod.get_activation_tables

    def _tables_ln_exp_combined(arch):
        # empty out every other set so ALL activation funcs resolve to the one
        # set holding ln+exp -> exactly one ACT_TABLE_LOAD in the whole kernel
        tabs = _orig_tables(arch)
        return {
            name: (fns if name == "natural_log_exp_and_others" else set())
            for name, fns in tabs.items()
        }

    _bacc_mod.get_activation_tables = _tables_ln_exp_combined
    try:
        nc.compile()
    finally:
        _bacc_mod.get_activation_tables = _orig_tables
    _CACHE["nc"] = nc
    return nc


def _pk16(wT):
    """[C, 256] (c, o) weight -> pack [128, 2, 256] fp16."""
    return np.ascontiguousarray(
        np.asarray(wT, np.float32).reshape(2, 128, 256).transpose(1, 0, 2)
    ).astype(np.float16)


def kernel(x_ri, gn_gamma, gn_beta, qw, qb, kw, kb, vw, vb, pw, pb):
    x_ri = np.asarray(x_ri, np.float32)
    f = lambda a: np.ascontiguousarray(np.asarray(a, np.float32))
    qw, qb = np.asarray(qw, np.float32), np.asarray(qb, np.float32)
    kw, kb = np.asarray(kw, np.float32), np.asarray(kb, np.float32)
    vw, vb = np.asarray(vw, np.float32), np.asarray(vb, np.float32)
    pw, pb = np.asarray(pw, np.float32), np.asarray(pb, np.float32)
    s4 = float(C) ** -0.25

    common = {
        "wqr": _pk16(qw[0].T * s4), "wqi": _pk16(qw[1].T * s4), "nwqi": _pk16(-qw[1].T * s4),
        "wkr": _pk16(kw[0].T * s4), "wki": _pk16(kw[1].T * s4), "nwki": _pk16(-kw[1].T * s4),
        "wvr": _pk16(vw[0].T), "wvi": _pk16(vw[1].T), "nwvi": _pk16(-vw[1].T),
        "wpr": _pk16(pw[0].T), "wpi": _pk16(pw[1].T), "nwpi": _pk16(-pw[1].T),
        "qbr": f(qb[0] * s4).reshape(C, 1), "qbi": f(qb[1] * s4).reshape(C, 1),
        "nqbi": f(-qb[1] * s4).reshape(C, 1),
        "kbr": f(kb[0] * s4).reshape(C, 1), "kbi": f(kb[1] * s4).reshape(C, 1),
        "bvr": np.ascontiguousarray(vb[0].reshape(1, C).astype(np.float16)),
        "bvi": np.ascontiguousarray(vb[1].reshape(1, C).astype(np.float16)),
        "nbvi": np.ascontiguousarray((-vb[1]).reshape(1, C).astype(np.float16)),
        "gamma_r": f(gn_gamma[0::2]).reshape(C, 1), "gamma_i": f(gn_gamma[1::2]).reshape(C, 1),
        "beta_r": f(gn_beta[0::2]).reshape(C, 1), "beta_i": f(gn_beta[1::2]).reshape(C, 1),
        "selmat": np.eye(16, dtype=np.float32)[np.arange(128) // 8],
        "expmat": np.ascontiguousarray(np.eye(16, dtype=np.float32)[np.arange(128) // 8].T),
        "ones_row": np.ones((1, 128), np.float16),
    }
    ones8 = np.zeros((128, 2, 16), np.float32)
    ones8[:, :, 0] = 1.0
    common["ones8"] = np.ascontiguousarray(ones8.astype(ml_dtypes.float8_e4m3))

    xr = np.ascontiguousarray(x_ri[..., 0].reshape(B, C, N))
    xi = np.ascontiguousarray(x_ri[..., 1].reshape(B, C, N))

    def pack16(a):  # [C, N] -> [128, 2, N] fp16
        return np.ascontiguousarray(a.reshape(2, 128, N).transpose(1, 0, 2).astype(np.float16))

    pbr_col = pb[0].reshape(C, 1)
    pbi_col = pb[1].reshape(C, 1)
    in_maps = []
    for core in range(8):
        b, half = core // 2, core % 2
        q0 = half * NQ
        xrr = np.ascontiguousarray(np.roll(xr[b], -q0, axis=1))
        xir = np.ascontiguousarray(np.roll(xi[b], -q0, axis=1))
        in_maps.append({
            **common,
            "x16r": pack16(xrr), "x16i": pack16(xir),
            "xr32": np.ascontiguousarray(xrr + pbr_col),
            "xi32": np.ascontiguousarray(xir + pbi_col),
        })

    nc = _build()
    trace = os.environ.get("BASS_KERNEL_TRACE") == "1"
    res = run_bass_kernel_spmd(nc, in_maps, core_ids=list(range(8)), trace=trace)
    kernel._last_result = res

    out = np.empty((B, C, N), np.complex64)
    for core in range(8):
        b, half = core // 2, core % 2
        q0 = half * NQ
        rr = res.results[core]
        out[b, :, q0:q0 + NQ] = rr["outr"] + 1j * rr["outi"]
    return out.reshape(B, C, HH, WW)


kernel._last_result = None


# revision 3
# speedup vs baseline: 1.0015x; 1.0015x over previous
"""Trainium2 Bass kernel for a complex-valued attention block — v3.

Reference computation (per batch sample, complex64):
  h = ComplexGroupNorm(x); q,k,v = 1x1 complex convs of h
  attn = (q^T k) * C^-0.5; aw = magnitude-softmax(attn)
  out = v @ aw^T; y = x + 1x1conv_p(out)

Sharding: 8 cores = 4 batches x 2 query-halves (x rolled so each core's
query block is columns [0, 1152)).

v3 (vs the two-stage v1):
  - one-pass softmax per group of SG=3 key tiles with LAGGED v-matmul
    emission: the PE queue runs group g+2's score matmuls while group g's
    ln/exp chain completes, so the tensor engine never waits on softmax.
  - all matmuls fp16 (fp8 scores fail: the peaked softmax amplifies
    quantization; measured 3e-2 rel err) except the denominator, which is
    an fp8 DoubleRow ones-matmul over te pairs (te8 errs only ~2e-3).
  - EXP_SHIFT 2^-8 keeps te8 under the fp8e4 max (peak |attn| is 10.9).
  - e^mag via a 4th ACT pass writing fp8 directly; 1/den = exp(-ln(den))
    on ACT straight from PSUM; negated vT / negated q_i copies instead of
    per-chunk negations; all softmax intermediates fp16 (lbias=1e-8 keeps
    them in range); p-bias folded into the residual on the host.
"""

import os
import ml_dtypes
import numpy as np

import concourse.bacc as bacc
import concourse.bass as bass
import concourse.tile as tile
from concourse import mybir
from concourse.bass_utils import run_bass_kernel_spmd

B, C, HH, WW = 4, 256, 48, 48
N = HH * WW            # 2304 keys
NQ = N // 2            # 1152 queries per core
CK = 384               # query-chunk width
NCK = NQ // CK         # 3 chunks
MT = N // 128          # 18 key tiles
SG = 3
NG = MT // SG          # 6 softmax groups per chunk
LAG = 2                # groups between score emission and v-matmul emission
GN_EPS = 1e-6

F32 = mybir.dt.float32
FP16 = mybir.dt.float16
FP8 = mybir.dt.float8e4
EXP_SHIFT = -5.545177444479562  # -8*ln2; cancels via the denominator
LBIAS = 1e-8

AF = mybir.ActivationFunctionType
OP = mybir.AluOpType
DRM = mybir.MatmulPerfMode.DoubleRow


def _emit(nc, tc, d):
    def pool(name, bufs, space="SBUF"):
        return tc.tile_pool(name=name, bufs=bufs, space=space)

    def mm(out, lhsT, rhs, start, stop):
        nc.tensor.matmul(out, lhsT, rhs, start=start, stop=stop,
                         skip_group_check=True)

    with pool("const", 1) as const, pool("persist", 1) as pers:

        # ---- x first: GN stats are the critical path at kernel start ----
        x16 = {p_: pers.tile([128, 2, N], FP16, tag=f"x16{p_}", name=f"x16{p_}")
               for p_ in ("r", "i")}
        for p_ in ("r", "i"):
            for t in (0, 1):
                nc.sync.dma_start(out=x16[p_][:, t, :],
                                  in_=d["x16r" if p_ == "r" else "x16i"][:, t, :])

        # ---- GN-chain consts, then ONE packed vecs DMA (each dma_start costs
        # ~600ns of sync-queue issue time; 26 tiny vector DMAs would delay the
        # weight packs by >10us) ----
        selmat = const.tile([128, 16], F32, tag="selmat", name="selmat")
        nc.sync.dma_start(out=selmat, in_=d["selmat"])
        expmat = const.tile([16, 128], F32, tag="expmat", name="expmat")
        nc.sync.dma_start(out=expmat, in_=d["expmat"])
        vecs = const.tile([128, 2, 9], F32, tag="vecs", name="vecs")
        nc.sync.dma_start(out=vecs, in_=d["vecs"])
        VIDX = {"qbr": 0, "qbi": 1, "nqbi": 2, "kbr": 3, "kbi": 4,
                "gam_r": 5, "gam_i": 6, "bet_r": 7, "bet_i": 8}
        gam = {(p_, t): vecs[:, t, VIDX[f"gam_{p_}"]:VIDX[f"gam_{p_}"] + 1]
               for p_ in ("r", "i") for t in (0, 1)}
        bet = {(p_, t): vecs[:, t, VIDX[f"bet_{p_}"]:VIDX[f"bet_{p_}"] + 1]
               for p_ in ("r", "i") for t in (0, 1)}
        qkb = {(nm, co): vecs[:, co, VIDX[nm]:VIDX[nm] + 1]
               for nm in ("qbr", "qbi", "nqbi", "kbr", "kbi") for co in (0, 1)}

        # ---- weight packs (needed once projections start) ----
        wpk = {}
        for nm in ("wqr", "wqi", "nwqi", "wkr", "wki", "nwki",
                   "wvr", "wvi", "nwvi", "wpr", "wpi", "nwpi"):
            t = const.tile([128, 2, 256], FP16, tag=nm, name=nm)
            nc.sync.dma_start(out=t, in_=d[nm])
            wpk[nm] = t

        bvrow = {}
        for nm in ("bvr", "bvi", "nbvi"):
            t = const.tile([1, 256], FP16, tag=nm, name=nm)
            nc.sync.dma_start(out=t, in_=d[nm])
            bvrow[nm] = t
        ones8 = const.tile([128, 2, 16], FP8, tag="ones8", name="ones8")
        nc.sync.dma_start(out=ones8, in_=d["ones8"])
        ones_row = const.tile([1, 128], FP16, tag="ones_row", name="ones_row")
        nc.sync.dma_start(out=ones_row, in_=d["ones_row"])
        ebias = const.tile([128, 1], F32, tag="ebias", name="ebias")
        nc.vector.memset(ebias, EXP_SHIFT)
        lbias = const.tile([128, 1], F32, tag="lbias", name="lbias")
        nc.vector.memset(lbias, LBIAS)

        # ---- persistent activations ----
        h16 = {p_: pers.tile([128, 2, N], FP16, tag=f"h16{p_}", name=f"h16{p_}")
               for p_ in ("r", "i")}
        q16 = {nm: pers.tile([128, 2, NQ], FP16, tag=f"q16{nm}", name=f"q16{nm}")
               for nm in ("r", "i", "ni")}
        k16 = {nm: pers.tile([128, 2, N], FP16, tag=f"k16{nm}", name=f"k16{nm}")
               for nm in ("r", "i")}
        vT = {nm: pers.tile([128, MT, 256], FP16, tag=f"vT{nm}", name=f"vT{nm}")
              for nm in ("r", "i", "ni")}

        # ================= phase 1: GroupNorm -> h16 packs =================
        with pool("gnw", 2) as gnw, pool("ps_small", 1, "PSUM") as ps_small, \
             nc.named_scope("groupnorm"):
            mv = {}
            for p_ in ("r", "i"):
                for t in (0, 1):
                    xv = x16[p_][:, t, :].rearrange("p (s f) -> p s f", f=256)
                    st = gnw.tile([128, 9, 6], F32, tag="bnstats", name="bnstats")
                    for s9 in range(9):
                        nc.vector.bn_stats(out=st[:, s9, :], in_=xv[:, s9, :])
                    m = gnw.tile([128, 2], F32, tag=f"mv{p_}{t}", name=f"mv{p_}{t}", bufs=1)
                    nc.vector.bn_aggr(out=m, in_=st)
                    mv[(p_, t)] = m
            # merged t=0/1 aggregation: srhs [128, 8] = [t, part, (mean, E2)]
            srhs = gnw.tile([128, 8], F32, tag="srhs", name="srhs", bufs=1)
            for t in (0, 1):
                for ci, p_ in enumerate(("r", "i")):
                    m = mv[(p_, t)]
                    o = 4 * t + 2 * ci
                    nc.vector.tensor_copy(out=srhs[:, o:o + 1], in_=m[:, 0:1])
                    tmp = gnw.tile([128, 1], F32, tag="gtmp", name="gtmp")
                    nc.vector.tensor_tensor(out=tmp, in0=m[:, 0:1], in1=m[:, 0:1], op=OP.mult)
                    nc.vector.tensor_tensor(out=srhs[:, o + 1:o + 2],
                                            in0=tmp, in1=m[:, 1:2], op=OP.add)
            psg = ps_small.tile([16, 8], F32, tag="psg", name="psg")
            nc.tensor.matmul(psg, selmat, srhs, start=True, stop=True)
            gsum = gnw.tile([16, 8], F32, tag="gsum", name="gsum", bufs=1)
            nc.scalar.copy(out=gsum, in_=psg)
            gv = gsum[:].rearrange("p (t c) -> p t c", c=4)
            # st4 [16, (mu_t0, rstd_t0, mu_t1, rstd_t1)]
            st4 = gnw.tile([16, 2, 2], F32, tag="st4", name="st4", bufs=1)
            mu = st4[:, :, 0:1]
            nc.vector.tensor_tensor(out=mu, in0=gv[:, :, 0:1], in1=gv[:, :, 2:3], op=OP.add)
            nc.vector.tensor_scalar_mul(mu, mu, 1.0 / 16.0)
            ex2 = gnw.tile([16, 2], F32, tag="gex2", name="gex2")
            nc.vector.tensor_tensor(out=ex2, in0=gv[:, :, 1:2], in1=gv[:, :, 3:4], op=OP.add)
            nc.vector.tensor_scalar_mul(ex2, ex2, 1.0 / 16.0)
            mu2 = gnw.tile([16, 2], F32, tag="gmu2", name="gmu2")
            nc.vector.tensor_tensor(out=mu2, in0=mu[:, :, 0], in1=mu[:, :, 0], op=OP.mult)
            var = gnw.tile([16, 2], F32, tag="gvar", name="gvar")
            nc.vector.tensor_tensor(out=var, in0=ex2, in1=mu2, op=OP.subtract)
            epst = gnw.tile([16, 1], F32, tag="geps", name="geps", bufs=1)
            nc.vector.memset(epst, GN_EPS)
            lnv = gnw.tile([16, 2], F32, tag="glnv", name="glnv")
            nc.scalar.activation(out=lnv, in_=var, func=AF.Ln, bias=epst, scale=1.0)
            nc.scalar.activation(out=st4[:, :, 1:2], in_=lnv, func=AF.Exp, bias=0.0, scale=-0.5)
            psr = ps_small.tile([128, 4], F32, tag="psr", name="psr")
            nc.tensor.matmul(psr, expmat, st4, start=True, stop=True)
            gst = gnw.tile([128, 2, 2], F32, tag="gst", name="gst", bufs=1)
            nc.scalar.copy(out=gst, in_=psr)
            for p_ in ("r", "i"):
                for t in (0, 1):
                    sc = gnw.tile([128, 1], F32, tag=f"sc{p_}{t}", name=f"sc{p_}{t}", bufs=1)
                    nc.vector.tensor_tensor(out=sc, in0=gst[:, t, 1:2], in1=gam[(p_, t)], op=OP.mult)
                    bi = gnw.tile([128, 1], F32, tag=f"bi{p_}{t}", name=f"bi{p_}{t}", bufs=1)
                    nc.vector.tensor_tensor(out=bi, in0=gst[:, t, 0:1], in1=sc, op=OP.mult)
                    nc.vector.scalar_tensor_tensor(out=bi, in0=bi, scalar=-1.0,
                                                   in1=bet[(p_, t)], op0=OP.mult, op1=OP.add)
                    # apply on DVE (fp16 tensor_scalar runs 4x; ACT pays 2.2us/tile)
                    nc.vector.tensor_scalar(out=h16[p_][:, t, :], in0=x16[p_][:, t, :],
                                            scalar1=sc, scalar2=bi, op0=OP.mult, op1=OP.add)

        # ============ phase 2: q/k/vT projections (fp16) ============
        with pool("ps_proj", 4, "PSUM") as psp, nc.named_scope("qkv"):
            for ic in range(NCK):
                cols = slice(ic * CK, (ic + 1) * CK)
                for co in (0, 1):
                    co_sl = slice(co * 128, (co + 1) * 128)
                    ps_r = psp.tile([128, CK], F32, tag="pp", name="ps_qr")
                    ps_i = psp.tile([128, CK], F32, tag="pp", name="ps_qi")
                    for t in (0, 1):
                        mm(ps_r, wpk["wqr"][:, t, co_sl], h16["r"][:, t, cols], t == 0, False)
                        mm(ps_i, wpk["wqr"][:, t, co_sl], h16["i"][:, t, cols], t == 0, False)
                    for t in (0, 1):
                        mm(ps_r, wpk["nwqi"][:, t, co_sl], h16["i"][:, t, cols], False, t == 1)
                        mm(ps_i, wpk["wqi"][:, t, co_sl], h16["r"][:, t, cols], False, t == 1)
                    nc.scalar.activation(out=q16["r"][:, co, cols], in_=ps_r,
                                         func=AF.Identity, bias=qkb[("qbr", co)], scale=1.0)
                    nc.scalar.activation(out=q16["i"][:, co, cols], in_=ps_i,
                                         func=AF.Identity, bias=qkb[("qbi", co)], scale=1.0)
                    nc.vector.tensor_scalar(out=q16["ni"][:, co, cols], in0=ps_i,
                                            scalar1=-1.0, scalar2=qkb[("nqbi", co)],
                                            op0=OP.mult, op1=OP.add)
            for ic in range(N // CK):
                cols = slice(ic * CK, (ic + 1) * CK)
                for co in (0, 1):
                    co_sl = slice(co * 128, (co + 1) * 128)
                    ps_r = psp.tile([128, CK], F32, tag="pp", name="ps_kr")
                    ps_i = psp.tile([128, CK], F32, tag="pp", name="ps_ki")
                    for t in (0, 1):
                        mm(ps_r, wpk["wkr"][:, t, co_sl], h16["r"][:, t, cols], t == 0, False)
                        mm(ps_i, wpk["wkr"][:, t, co_sl], h16["i"][:, t, cols], t == 0, False)
                    for t in (0, 1):
                        mm(ps_r, wpk["nwki"][:, t, co_sl], h16["i"][:, t, cols], False, t == 1)
                        mm(ps_i, wpk["wki"][:, t, co_sl], h16["r"][:, t, cols], False, t == 1)
                    nc.vector.tensor_scalar_add(k16["r"][:, co, cols], ps_r, qkb[("kbr", co)])
                    nc.vector.tensor_scalar_add(k16["i"][:, co, cols], ps_i, qkb[("kbi", co)])
            # bv broadcast emitted here (not at kernel start) so the tiny
            # matmuls don't block the GN aggregation matmul in the PE queue
            bv_bc = {}
            for nm in ("bvr", "bvi", "nbvi"):
                psb = psp.tile([128, 256], F32, tag="pp", name="psb")
                nc.tensor.matmul(psb, ones_row[:], bvrow[nm][:], start=True, stop=True)
                t = const.tile([128, 256], F32, tag=nm + "_bc", name=nm + "_bc")
                nc.scalar.copy(out=t, in_=psb)
                bv_bc[nm] = t
            with nc.named_scope("vproj"):
                for j in range(MT):
                    msl = slice(j * 128, (j + 1) * 128)
                    ps_vr = psp.tile([128, 256], F32, tag="pp", name="ps_vr")
                    ps_vi = psp.tile([128, 256], F32, tag="pp", name="ps_vi")
                    for t in (0, 1):
                        mm(ps_vr, h16["r"][:, t, msl], wpk["wvr"][:, t, :], t == 0, False)
                        mm(ps_vi, h16["r"][:, t, msl], wpk["wvi"][:, t, :], t == 0, False)
                    for t in (0, 1):
                        mm(ps_vr, h16["i"][:, t, msl], wpk["nwvi"][:, t, :], False, t == 1)
                        mm(ps_vi, h16["i"][:, t, msl], wpk["wvr"][:, t, :], False, t == 1)
                    nc.vector.tensor_tensor(out=vT["r"][:, j, :], in0=ps_vr, in1=bv_bc["bvr"], op=OP.add)
                    nc.vector.tensor_tensor(out=vT["i"][:, j, :], in0=ps_vi, in1=bv_bc["bvi"], op=OP.add)
                    nc.vector.scalar_tensor_tensor(out=vT["ni"][:, j, :], in0=ps_vi, scalar=-1.0,
                                                   in1=bv_bc["nbvi"], op0=OP.mult, op1=OP.add)

        # ============ phase 3: attention (one pass, lagged v-matmul) ============
        with pool("psA", 3, "PSUM") as psA, pool("psACC", 1, "PSUM") as psACC, \
             pool("psD", 1, "PSUM") as psD, \
             pool("cpp", 2) as cpp, pool("smp", 2) as smp, pool("awp", LAG + 1) as awp, \
             pool("ckp", 2) as ckp:
            # per-chunk state, created lazily as the global group stream reaches it
            st = {}

            def chunk_state(ic):
                if ic not in st:
                    st[ic] = {
                        "te8": awp.tile([128, MT, CK], FP8, tag="te8", name="te8", bufs=2),
                        "psden": psD.tile([128, 512], F32, tag="den", name="psden"),
                        "accs": None,
                        "aw": {},
                        "den_done": 0,
                    }
                return st[ic]

            def attn_group(ic, g):
                cols = slice(ic * CK, (ic + 1) * CK)
                s = chunk_state(ic)
                cp = cpp.tile([128, SG, 2, CK], FP16, tag="cp", name="cp")
                for js in range(SG):
                    j = g * SG + js
                    msl = slice(j * 128, (j + 1) * 128)
                    ps_r = psA.tile([128, 512], F32, tag="pair", name="ps_r")
                    ps_i = psA.tile([128, 512], F32, tag="pair", name="ps_i")
                    for t in (0, 1):
                        mm(ps_r[:, 0:CK], k16["r"][:, t, msl], q16["r"][:, t, cols], t == 0, False)
                        mm(ps_i[:, 0:CK], k16["r"][:, t, msl], q16["i"][:, t, cols], t == 0, False)
                    for t in (0, 1):
                        mm(ps_r[:, 0:CK], k16["i"][:, t, msl], q16["ni"][:, t, cols], False, t == 1)
                        mm(ps_i[:, 0:CK], k16["i"][:, t, msl], q16["r"][:, t, cols], False, t == 1)
                    if js % 2 == 0:
                        nc.scalar.copy(out=cp[:, js, 0, :], in_=ps_r[:, 0:CK])
                        nc.vector.tensor_copy(out=cp[:, js, 1, :], in_=ps_i[:, 0:CK])
                    else:
                        nc.vector.tensor_copy(out=cp[:, js, 0, :], in_=ps_r[:, 0:CK])
                        nc.scalar.copy(out=cp[:, js, 1, :], in_=ps_i[:, 0:CK])
                sq = smp.tile([128, SG, 2, CK], FP16, tag="sq", name="sq")
                nc.vector.tensor_tensor(out=sq, in0=cp, in1=cp, op=OP.mult)
                m2 = smp.tile([128, SG, CK], FP16, tag="m2", name="m2")
                nc.vector.tensor_tensor(out=m2, in0=sq[:, :, 0, :], in1=sq[:, :, 1, :], op=OP.add)
                lf = smp.tile([128, SG, CK], FP16, tag="lf", name="lf")
                nc.scalar.activation(out=lf, in_=m2, func=AF.Ln, bias=lbias)
                mag = smp.tile([128, SG, CK], FP16, tag="mag", name="mag")
                nc.scalar.activation(out=mag, in_=lf, func=AF.Exp, bias=0.0, scale=0.5)
                u = smp.tile([128, SG, CK], FP16, tag="u", name="u")
                nc.vector.scalar_tensor_tensor(out=u, in0=lf, scalar=-0.5,
                                               in1=mag, op0=OP.mult, op1=OP.add)
                tt = smp.tile([128, SG, CK], FP16, tag="tt", name="tt")
                nc.scalar.activation(out=tt, in_=u, func=AF.Exp, bias=ebias)
                nc.scalar.activation(out=s["te8"][:, g * SG:(g + 1) * SG, :], in_=mag,
                                     func=AF.Exp, bias=ebias)
                aw = awp.tile([128, SG, 2, CK], FP16, tag="aw", name="aw")
                nc.vector.tensor_tensor(out=aw[:, :, 0, :], in0=tt, in1=cp[:, :, 0, :], op=OP.mult)
                nc.vector.tensor_tensor(out=aw[:, :, 1, :], in0=tt, in1=cp[:, :, 1, :], op=OP.mult)
                s["aw"][g] = aw

            def vmm_group(ic, g):
                s = chunk_state(ic)
                if s["accs"] is None:
                    s["accs"] = {(p_, co): psACC.tile([128, CK], F32, tag=f"acc{p_}{co}",
                                                      name=f"acc{p_}{co}")
                                 for p_ in ("r", "i") for co in (0, 1)}
                accs = s["accs"]
                aw = s["aw"].pop(g)
                for js in range(SG):
                    j = g * SG + js
                    st_ = (j == 0)
                    sp_ = (j == MT - 1)
                    for co in (0, 1):
                        co_sl = slice(co * 128, (co + 1) * 128)
                        nc.tensor.matmul(accs[("r", co)], vT["r"][:, j, co_sl], aw[:, js, 0, :], start=st_, stop=False, skip_group_check=True)
                        nc.tensor.matmul(accs[("i", co)], vT["r"][:, j, co_sl], aw[:, js, 1, :], start=st_, stop=False, skip_group_check=True)
                        nc.tensor.matmul(accs[("r", co)], vT["ni"][:, j, co_sl], aw[:, js, 1, :], start=False, stop=sp_, skip_group_check=True)
                        nc.tensor.matmul(accs[("i", co)], vT["i"][:, j, co_sl], aw[:, js, 0, :], start=False, stop=sp_, skip_group_check=True)
                hi = ((g + 1) * SG) // 2
                for jp in range(s["den_done"], hi):
                    nc.tensor.matmul(s["psden"][0:1, 0:CK], ones8[:, :, 0:1],
                                     s["te8"][:, 2 * jp:2 * jp + 2, :],
                                     start=(jp == 0), stop=(jp == MT // 2 - 1),
                                     perf_mode=DRM, skip_group_check=True)
                s["den_done"] = hi

            def tail(ic):
                cols = slice(ic * CK, (ic + 1) * CK)
                s = st.pop(ic)
                psden, accs = s["psden"], s["accs"]
                lden = ckp.tile([1, CK], F32, tag="lden", name="lden")
                nc.scalar.activation(out=lden, in_=psden[0:1, 0:CK], func=AF.Ln, bias=0.0)
                rec16 = ckp.tile([1, CK], FP16, tag="rec16", name="rec16")
                nc.scalar.activation(out=rec16, in_=lden, func=AF.Exp, bias=0.0, scale=-1.0)
                nc.tensor.matmul(psden[:, 0:CK], ones_row[:], rec16[:], start=True, stop=True, skip_group_check=True)
                sden = ckp.tile([128, CK], FP16, tag="sden", name="sden")
                nc.vector.tensor_copy(out=sden, in_=psden[:, 0:CK])

                outsc = {p_: ckp.tile([128, 2, CK], FP16, tag=f"osc{p_}", name=f"osc{p_}")
                         for p_ in ("r", "i")}
                for p_ in ("r", "i"):
                    for co in (0, 1):
                        nc.vector.tensor_tensor(out=outsc[p_][:, co, :], in0=accs[(p_, co)],
                                                in1=sden, op=OP.mult)

                for co in (0, 1):
                    co_sl = slice(co * 128, (co + 1) * 128)
                    ps_pr = psACC.tile([128, CK], F32, tag="accr0", name="ps_pr")
                    ps_pi = psACC.tile([128, CK], F32, tag="acci0", name="ps_pi")
                    for t in (0, 1):
                        mm(ps_pr, wpk["wpr"][:, t, co_sl], outsc["r"][:, t, :], t == 0, False)
                        mm(ps_pi, wpk["wpr"][:, t, co_sl], outsc["i"][:, t, :], t == 0, False)
                    for t in (0, 1):
                        mm(ps_pr, wpk["nwpi"][:, t, co_sl], outsc["i"][:, t, :], False, t == 1)
                        mm(ps_pi, wpk["wpi"][:, t, co_sl], outsc["r"][:, t, :], False, t == 1)
                    for p_, ps_p in (("r", ps_pr), ("i", ps_pi)):
                        # xr32/xi32 carry x + p-bias (host-folded)
                        xres = ckp.tile([128, CK], F32, tag=f"xres{p_}{co}",
                                        name=f"xres{p_}{co}", bufs=1)
                        nc.sync.dma_start(out=xres, in_=d["xr32" if p_ == "r" else "xi32"][co * 128:(co + 1) * 128, cols])
                        nc.vector.scalar_tensor_tensor(out=xres, in0=ps_p, scalar=1.0,
                                                       in1=xres, op0=OP.mult, op1=OP.add)
                        nc.sync.dma_start(out=d["outr" if p_ == "r" else "outi"][co * 128:(co + 1) * 128, cols], in_=xres)

            # global group stream: chunk ic's tail interleaves with chunk
            # ic+1's first attention groups, so the PE never idles on the
            # denominator/outsc/p-proj chain.
            TOT = NCK * NG
            for gg in range(TOT + LAG):
                ic, g = divmod(gg, NG)
                if gg < TOT:
                    with nc.named_scope(f"attn_chunk{ic}"):
                        attn_group(ic, g)
                vg = gg - LAG
                if vg >= 0:
                    vic, vgg = divmod(vg, NG)
                    with nc.named_scope(f"attn_chunk{vic}"):
                        vmm_group(vic, vgg)
                        if vgg == NG - 1:
                            tail(vic)


_CACHE = {}


def _build():
    if "nc" in _CACHE:
        return _CACHE["nc"]
    nc = bacc.Bacc("TRN2", target_bir_lowering=False, debug=False, num_devices=8)
    d = {}
    ins = {
        "x16r": ([128, 2, N], FP16), "x16i": ([128, 2, N], FP16),
        "xr32": ([C, N], F32), "xi32": ([C, N], F32),
        "vecs": ([128, 2, 9], F32),
        "bvr": ([1, C], FP16), "bvi": ([1, C], FP16), "nbvi": ([1, C], FP16),
        "selmat": ([128, 16], F32), "expmat": ([16, 128], F32),
        "ones8": ([128, 2, 16], FP8), "ones_row": ([1, 128], FP16),
    }
    for nm in ("wqr", "wqi", "nwqi", "wkr", "wki", "nwki",
               "wvr", "wvi", "nwvi", "wpr", "wpi", "nwpi"):
        ins[nm] = ([128, 2, 256], FP16)
    for nm, (sh, dt_) in ins.items():
        d[nm] = nc.dram_tensor(nm, list(sh), dt_, kind="ExternalInput").ap()
    for nm in ("outr", "outi"):
        d[nm] = nc.dram_tensor(nm, [C, NQ], F32, kind="ExternalOutput").ap()
    with tile.TileContext(nc) as tc:
        _emit(nc, tc, d)
    import concourse.bacc as _bacc_mod
    _orig_tables = _bacc_mod.get_activation_tables

    def _tables_ln_exp_combined(arch):
        # empty out every other set so ALL activation funcs resolve to the one
        # set holding ln+exp -> exactly one ACT_TABLE_LOAD in the whole kernel
        tabs = _orig_tables(arch)
        return {
            name: (fns if name == "natural_log_exp_and_others" else set())
            for name, fns in tabs.items()
        }

    _bacc_mod.get_activation_tables = _tables_ln_exp_combined
    try:
        nc.compile()
    finally:
        _bacc_mod.get_activation_tables = _orig_tables
    _CACHE["nc"] = nc
    return nc


def _pk16(wT):
    """[C, 256] (c, o) weight -> pack [128, 2, 256] fp16."""
    return np.ascontiguousarray(
        np.asarray(wT, np.float32).reshape(2, 128, 256).transpose(1, 0, 2)
    ).astype(np.float16)


def kernel(x_ri, gn_gamma, gn_beta, qw, qb, kw, kb, vw, vb, pw, pb):
    x_ri = np.asarray(x_ri, np.float32)
    f = lambda a: np.ascontiguousarray(np.asarray(a, np.float32))
    qw, qb = np.asarray(qw, np.float32), np.asarray(qb, np.float32)
    kw, kb = np.asarray(kw, np.float32), np.asarray(kb, np.float32)
    vw, vb = np.asarray(vw, np.float32), np.asarray(vb, np.float32)
    pw, pb = np.asarray(pw, np.float32), np.asarray(pb, np.float32)
    s4 = float(C) ** -0.25

    common = {
        "wqr": _pk16(qw[0].T * s4), "wqi": _pk16(qw[1].T * s4), "nwqi": _pk16(-qw[1].T * s4),
        "wkr": _pk16(kw[0].T * s4), "wki": _pk16(kw[1].T * s4), "nwki": _pk16(-kw[1].T * s4),
        "wvr": _pk16(vw[0].T), "wvi": _pk16(vw[1].T), "nwvi": _pk16(-vw[1].T),
        "wpr": _pk16(pw[0].T), "wpi": _pk16(pw[1].T), "nwpi": _pk16(-pw[1].T),
        "bvr": np.ascontiguousarray(vb[0].reshape(1, C).astype(np.float16)),
        "bvi": np.ascontiguousarray(vb[1].reshape(1, C).astype(np.float16)),
        "nbvi": np.ascontiguousarray((-vb[1]).reshape(1, C).astype(np.float16)),
        "selmat": np.eye(16, dtype=np.float32)[np.arange(128) // 8],
        "expmat": np.ascontiguousarray(np.eye(16, dtype=np.float32)[np.arange(128) // 8].T),
        "ones_row": np.ones((1, 128), np.float16),
    }
    ones8 = np.zeros((128, 2, 16), np.float32)
    ones8[:, :, 0] = 1.0
    common["ones8"] = np.ascontiguousarray(ones8.astype(ml_dtypes.float8_e4m3))
    # all [C]-shaped vectors in one DMA: vecs[p, t, idx], channel c = 128t+p
    vv = np.stack([qb[0] * s4, qb[1] * s4, -qb[1] * s4, kb[0] * s4, kb[1] * s4,
                   gn_gamma[0::2], gn_gamma[1::2], gn_beta[0::2], gn_beta[1::2]],
                  axis=1).astype(np.float32)          # [C, 9]
    common["vecs"] = np.ascontiguousarray(vv.reshape(2, 128, 9).transpose(1, 0, 2))

    xr = np.ascontiguousarray(x_ri[..., 0].reshape(B, C, N))
    xi = np.ascontiguousarray(x_ri[..., 1].reshape(B, C, N))

    def pack16(a):  # [C, N] -> [128, 2, N] fp16
        return np.ascontiguousarray(a.reshape(2, 128, N).transpose(1, 0, 2).astype(np.float16))

    pbr_col = pb[0].reshape(C, 1)
    pbi_col = pb[1].reshape(C, 1)
    in_maps = []
    for core in range(8):
        b, half = core // 2, core % 2
        q0 = half * NQ
        xrr = np.ascontiguousarray(np.roll(xr[b], -q0, axis=1))
        xir = np.ascontiguousarray(np.roll(xi[b], -q0, axis=1))
        in_maps.append({
            **common,
            "x16r": pack16(xrr), "x16i": pack16(xir),
            "xr32": np.ascontiguousarray(xrr + pbr_col),
            "xi32": np.ascontiguousarray(xir + pbi_col),
        })

    nc = _build()
    trace = os.environ.get("BASS_KERNEL_TRACE") == "1"
    res = run_bass_kernel_spmd(nc, in_maps, core_ids=list(range(8)), trace=trace)
    kernel._last_result = res

    out = np.empty((B, C, N), np.complex64)
    for core in range(8):
        b, half = core // 2, core % 2
        q0 = half * NQ
        rr = res.results[core]
        out[b, :, q0:q0 + NQ] = rr["outr"] + 1j * rr["outi"]
    return out.reshape(B, C, HH, WW)


kernel._last_result = None


# revision 4
# speedup vs baseline: 1.0127x; 1.0113x over previous
"""Trainium2 Bass kernel for a complex-valued attention block — v3.

Reference computation (per batch sample, complex64):
  h = ComplexGroupNorm(x); q,k,v = 1x1 complex convs of h
  attn = (q^T k) * C^-0.5; aw = magnitude-softmax(attn)
  out = v @ aw^T; y = x + 1x1conv_p(out)

Sharding: 8 cores = 4 batches x 2 query-halves (x rolled so each core's
query block is columns [0, 1152)).

v3 (vs the two-stage v1):
  - one-pass softmax per group of SG=3 key tiles with LAGGED v-matmul
    emission: the PE queue runs group g+2's score matmuls while group g's
    ln/exp chain completes, so the tensor engine never waits on softmax.
  - all matmuls fp16 (fp8 scores fail: the peaked softmax amplifies
    quantization; measured 3e-2 rel err) except the denominator, which is
    an fp8 DoubleRow ones-matmul over te pairs (te8 errs only ~2e-3).
  - EXP_SHIFT 2^-8 keeps te8 under the fp8e4 max (peak |attn| is 10.9).
  - e^mag via a 4th ACT pass writing fp8 directly; 1/den = exp(-ln(den))
    on ACT straight from PSUM; negated vT / negated q_i copies instead of
    per-chunk negations; all softmax intermediates fp16 (lbias=1e-8 keeps
    them in range); p-bias folded into the residual on the host.
"""

import os
import ml_dtypes
import numpy as np

import concourse.bacc as bacc
import concourse.bass as bass
import concourse.tile as tile
from concourse import mybir
from concourse.bass_utils import run_bass_kernel_spmd

B, C, HH, WW = 4, 256, 48, 48
N = HH * WW            # 2304 keys
NQ = N // 2            # 1152 queries per core
CK = 384               # query-chunk width
NCK = NQ // CK         # 3 chunks
MT = N // 128          # 18 key tiles
SG = 3
NG = MT // SG          # 6 softmax groups per chunk
LAG = 2                # groups between score emission and v-matmul emission
GN_EPS = 1e-6

F32 = mybir.dt.float32
FP16 = mybir.dt.float16
FP8 = mybir.dt.float8e4
EXP_SHIFT = -5.545177444479562  # -8*ln2; cancels via the denominator
LBIAS = 1e-8

AF = mybir.ActivationFunctionType
OP = mybir.AluOpType
DRM = mybir.MatmulPerfMode.DoubleRow


def _emit(nc, tc, d):
    def pool(name, bufs, space="SBUF"):
        return tc.tile_pool(name=name, bufs=bufs, space=space)

    def mm(out, lhsT, rhs, start, stop):
        nc.tensor.matmul(out, lhsT, rhs, start=start, stop=stop,
                         skip_group_check=True)

    with pool("const", 1) as const, pool("persist", 1) as pers:

        # ---- x first: GN stats are the critical path at kernel start ----
        x16 = {p_: pers.tile([128, 2, N], FP16, tag=f"x16{p_}", name=f"x16{p_}")
               for p_ in ("r", "i")}
        for p_ in ("r", "i"):
            for t in (0, 1):
                nc.sync.dma_start(out=x16[p_][:, t, :],
                                  in_=d["x16r" if p_ == "r" else "x16i"][:, t, :])

        # ---- GN-chain consts, then ONE packed vecs DMA (each dma_start costs
        # ~600ns of sync-queue issue time; 26 tiny vector DMAs would delay the
        # weight packs by >10us) ----
        selmat = const.tile([128, 16], F32, tag="selmat", name="selmat")
        nc.sync.dma_start(out=selmat, in_=d["selmat"])
        expmat = const.tile([16, 128], F32, tag="expmat", name="expmat")
        nc.sync.dma_start(out=expmat, in_=d["expmat"])
        vecs = const.tile([128, 2, 9], F32, tag="vecs", name="vecs")
        nc.sync.dma_start(out=vecs, in_=d["vecs"])
        VIDX = {"qbr": 0, "qbi": 1, "nqbi": 2, "kbr": 3, "kbi": 4,
                "gam_r": 5, "gam_i": 6, "bet_r": 7, "bet_i": 8}
        gam = {(p_, t): vecs[:, t, VIDX[f"gam_{p_}"]:VIDX[f"gam_{p_}"] + 1]
               for p_ in ("r", "i") for t in (0, 1)}
        bet = {(p_, t): vecs[:, t, VIDX[f"bet_{p_}"]:VIDX[f"bet_{p_}"] + 1]
               for p_ in ("r", "i") for t in (0, 1)}
        qkb = {(nm, co): vecs[:, co, VIDX[nm]:VIDX[nm] + 1]
               for nm in ("qbr", "qbi", "nqbi", "kbr", "kbi") for co in (0, 1)}

        # ---- weight packs (needed once projections start) ----
        wpk = {}
        for nm in ("wqr", "wqi", "nwqi", "wkr", "wki", "nwki",
                   "wvr", "wvi", "nwvi", "wpr", "wpi", "nwpi"):
            t = const.tile([128, 2, 256], FP16, tag=nm, name=nm)
            nc.sync.dma_start(out=t, in_=d[nm])
            wpk[nm] = t

        bvrow = {}
        for nm in ("bvr", "bvi", "nbvi"):
            t = const.tile([1, 256], FP16, tag=nm, name=nm)
            nc.sync.dma_start(out=t, in_=d[nm])
            bvrow[nm] = t
        ones8 = const.tile([128, 2, 16], FP8, tag="ones8", name="ones8")
        nc.sync.dma_start(out=ones8, in_=d["ones8"])
        ones_row = const.tile([1, 128], FP16, tag="ones_row", name="ones_row")
        nc.sync.dma_start(out=ones_row, in_=d["ones_row"])
        ebias = const.tile([128, 1], F32, tag="ebias", name="ebias")
        nc.vector.memset(ebias, EXP_SHIFT)
        lbias = const.tile([128, 1], F32, tag="lbias", name="lbias")
        nc.vector.memset(lbias, LBIAS)

        # ---- persistent activations ----
        h16 = {p_: pers.tile([128, 2, N], FP16, tag=f"h16{p_}", name=f"h16{p_}")
               for p_ in ("r", "i")}
        q16 = {nm: pers.tile([128, 2, NQ], FP16, tag=f"q16{nm}", name=f"q16{nm}")
               for nm in ("r", "i", "ni")}
        k16 = {nm: pers.tile([128, 2, N], FP16, tag=f"k16{nm}", name=f"k16{nm}")
               for nm in ("r", "i")}
        vT = {nm: pers.tile([128, MT, 256], FP16, tag=f"vT{nm}", name=f"vT{nm}")
              for nm in ("r", "i", "ni")}

        # ================= phase 1: GroupNorm -> h16 packs =================
        with pool("gnw", 2) as gnw, pool("ps_small", 1, "PSUM") as ps_small, \
             nc.named_scope("groupnorm"):
            mv = {}
            for p_ in ("r", "i"):
                for t in (0, 1):
                    xt = x16[p_][:, t, :]
                    st = gnw.tile([128, 5, 6], F32, tag="bnstats", name="bnstats")
                    for s5 in range(4):
                        nc.vector.bn_stats(out=st[:, s5, :], in_=xt[:, s5 * 512:(s5 + 1) * 512])
                    nc.vector.bn_stats(out=st[:, 4, :], in_=xt[:, 2048:2304])
                    m = gnw.tile([128, 2], F32, tag=f"mv{p_}{t}", name=f"mv{p_}{t}", bufs=1)
                    nc.vector.bn_aggr(out=m, in_=st)
                    mv[(p_, t)] = m
            # merged t=0/1 aggregation: srhs [128, 8] = [t, part, (mean, E2)]
            srhs = gnw.tile([128, 8], F32, tag="srhs", name="srhs", bufs=1)
            for t in (0, 1):
                for ci, p_ in enumerate(("r", "i")):
                    m = mv[(p_, t)]
                    o = 4 * t + 2 * ci
                    nc.vector.tensor_copy(out=srhs[:, o:o + 1], in_=m[:, 0:1])
                    tmp = gnw.tile([128, 1], F32, tag="gtmp", name="gtmp")
                    nc.vector.tensor_tensor(out=tmp, in0=m[:, 0:1], in1=m[:, 0:1], op=OP.mult)
                    nc.vector.tensor_tensor(out=srhs[:, o + 1:o + 2],
                                            in0=tmp, in1=m[:, 1:2], op=OP.add)
            psg = ps_small.tile([16, 8], F32, tag="psg", name="psg")
            nc.tensor.matmul(psg, selmat, srhs, start=True, stop=True)
            gsum = gnw.tile([16, 8], F32, tag="gsum", name="gsum", bufs=1)
            nc.scalar.copy(out=gsum, in_=psg)
            gv = gsum[:].rearrange("p (t c) -> p t c", c=4)
            # st4 [16, (mu_t0, rstd_t0, mu_t1, rstd_t1)]
            st4 = gnw.tile([16, 2, 2], F32, tag="st4", name="st4", bufs=1)
            mu = st4[:, :, 0:1]
            nc.vector.tensor_tensor(out=mu, in0=gv[:, :, 0:1], in1=gv[:, :, 2:3], op=OP.add)
            nc.vector.tensor_scalar_mul(mu, mu, 1.0 / 16.0)
            ex2 = gnw.tile([16, 2], F32, tag="gex2", name="gex2")
            nc.vector.tensor_tensor(out=ex2, in0=gv[:, :, 1:2], in1=gv[:, :, 3:4], op=OP.add)
            nc.vector.tensor_scalar_mul(ex2, ex2, 1.0 / 16.0)
            mu2 = gnw.tile([16, 2], F32, tag="gmu2", name="gmu2")
            nc.vector.tensor_tensor(out=mu2, in0=mu[:, :, 0], in1=mu[:, :, 0], op=OP.mult)
            var = gnw.tile([16, 2], F32, tag="gvar", name="gvar")
            nc.vector.tensor_tensor(out=var, in0=ex2, in1=mu2, op=OP.subtract)
            epst = gnw.tile([16, 1], F32, tag="geps", name="geps", bufs=1)
            nc.vector.memset(epst, GN_EPS)
            lnv = gnw.tile([16, 2], F32, tag="glnv", name="glnv")
            nc.scalar.activation(out=lnv, in_=var, func=AF.Ln, bias=epst, scale=1.0)
            nc.scalar.activation(out=st4[:, :, 1:2], in_=lnv, func=AF.Exp, bias=0.0, scale=-0.5)
            psr = ps_small.tile([128, 4], F32, tag="psr", name="psr")
            nc.tensor.matmul(psr, expmat, st4, start=True, stop=True)
            gst = gnw.tile([128, 2, 2], F32, tag="gst", name="gst", bufs=1)
            nc.scalar.copy(out=gst, in_=psr)
            for p_ in ("r", "i"):
                for t in (0, 1):
                    sc = gnw.tile([128, 1], F32, tag=f"sc{p_}{t}", name=f"sc{p_}{t}", bufs=1)
                    nc.vector.tensor_tensor(out=sc, in0=gst[:, t, 1:2], in1=gam[(p_, t)], op=OP.mult)
                    bi = gnw.tile([128, 1], F32, tag=f"bi{p_}{t}", name=f"bi{p_}{t}", bufs=1)
                    nc.vector.tensor_tensor(out=bi, in0=gst[:, t, 0:1], in1=sc, op=OP.mult)
                    nc.vector.scalar_tensor_tensor(out=bi, in0=bi, scalar=-1.0,
                                                   in1=bet[(p_, t)], op0=OP.mult, op1=OP.add)
                    # apply on DVE (fp16 tensor_scalar runs 4x; ACT pays 2.2us/tile)
                    nc.vector.tensor_scalar(out=h16[p_][:, t, :], in0=x16[p_][:, t, :],
                                            scalar1=sc, scalar2=bi, op0=OP.mult, op1=OP.add)

        # ============ phase 2: q/k/vT projections (fp16) ============
        with pool("ps_proj", 4, "PSUM") as psp, nc.named_scope("qkv"):
            for ic in range(NCK):
                cols = slice(ic * CK, (ic + 1) * CK)
                for co in (0, 1):
                    co_sl = slice(co * 128, (co + 1) * 128)
                    ps_r = psp.tile([128, CK], F32, tag="pp", name="ps_qr")
                    ps_i = psp.tile([128, CK], F32, tag="pp", name="ps_qi")
                    for t in (0, 1):
                        mm(ps_r, wpk["wqr"][:, t, co_sl], h16["r"][:, t, cols], t == 0, False)
                        mm(ps_i, wpk["wqr"][:, t, co_sl], h16["i"][:, t, cols], t == 0, False)
                    for t in (0, 1):
                        mm(ps_r, wpk["nwqi"][:, t, co_sl], h16["i"][:, t, cols], False, t == 1)
                        mm(ps_i, wpk["wqi"][:, t, co_sl], h16["r"][:, t, cols], False, t == 1)
                    nc.scalar.activation(out=q16["r"][:, co, cols], in_=ps_r,
                                         func=AF.Identity, bias=qkb[("qbr", co)], scale=1.0)
                    nc.scalar.activation(out=q16["i"][:, co, cols], in_=ps_i,
                                         func=AF.Identity, bias=qkb[("qbi", co)], scale=1.0)
                    nc.vector.tensor_scalar(out=q16["ni"][:, co, cols], in0=ps_i,
                                            scalar1=-1.0, scalar2=qkb[("nqbi", co)],
                                            op0=OP.mult, op1=OP.add)
            for ic in range(N // CK):
                cols = slice(ic * CK, (ic + 1) * CK)
                for co in (0, 1):
                    co_sl = slice(co * 128, (co + 1) * 128)
                    ps_r = psp.tile([128, CK], F32, tag="pp", name="ps_kr")
                    ps_i = psp.tile([128, CK], F32, tag="pp", name="ps_ki")
                    for t in (0, 1):
                        mm(ps_r, wpk["wkr"][:, t, co_sl], h16["r"][:, t, cols], t == 0, False)
                        mm(ps_i, wpk["wkr"][:, t, co_sl], h16["i"][:, t, cols], t == 0, False)
                    for t in (0, 1):
                        mm(ps_r, wpk["nwki"][:, t, co_sl], h16["i"][:, t, cols], False, t == 1)
                        mm(ps_i, wpk["wki"][:, t, co_sl], h16["r"][:, t, cols], False, t == 1)
                    nc.vector.tensor_scalar_add(k16["r"][:, co, cols], ps_r, qkb[("kbr", co)])
                    nc.vector.tensor_scalar_add(k16["i"][:, co, cols], ps_i, qkb[("kbi", co)])
            # bv broadcast emitted here (not at kernel start) so the tiny
            # matmuls don't block the GN aggregation matmul in the PE queue
            bv_bc = {}
            for nm in ("bvr", "bvi", "nbvi"):
                psb = psp.tile([128, 256], F32, tag="pp", name="psb")
                nc.tensor.matmul(psb, ones_row[:], bvrow[nm][:], start=True, stop=True)
                t = const.tile([128, 256], F32, tag=nm + "_bc", name=nm + "_bc")
                nc.scalar.copy(out=t, in_=psb)
                bv_bc[nm] = t
            with nc.named_scope("vproj"):
                for j in range(MT):
                    msl = slice(j * 128, (j + 1) * 128)
                    ps_vr = psp.tile([128, 256], F32, tag="pp", name="ps_vr")
                    ps_vi = psp.tile([128, 256], F32, tag="pp", name="ps_vi")
                    for t in (0, 1):
                        mm(ps_vr, h16["r"][:, t, msl], wpk["wvr"][:, t, :], t == 0, False)
                        mm(ps_vi, h16["r"][:, t, msl], wpk["wvi"][:, t, :], t == 0, False)
                    for t in (0, 1):
                        mm(ps_vr, h16["i"][:, t, msl], wpk["nwvi"][:, t, :], False, t == 1)
                        mm(ps_vi, h16["i"][:, t, msl], wpk["wvr"][:, t, :], False, t == 1)
                    nc.vector.tensor_tensor(out=vT["r"][:, j, :], in0=ps_vr, in1=bv_bc["bvr"], op=OP.add)
                    nc.vector.tensor_tensor(out=vT["i"][:, j, :], in0=ps_vi, in1=bv_bc["bvi"], op=OP.add)
                    nc.vector.scalar_tensor_tensor(out=vT["ni"][:, j, :], in0=ps_vi, scalar=-1.0,
                                                   in1=bv_bc["nbvi"], op0=OP.mult, op1=OP.add)

        # ============ phase 3: attention (one pass, lagged v-matmul) ============
        with pool("psA", 3, "PSUM") as psA, pool("psACC", 1, "PSUM") as psACC, \
             pool("psD", 1, "PSUM") as psD, \
             pool("cpp", 2) as cpp, pool("smp", 2) as smp, pool("awp", LAG + 1) as awp, \
             pool("ckp", 2) as ckp:
            # per-chunk state, created lazily as the global group stream reaches it
            st = {}

            def chunk_state(ic):
                if ic not in st:
                    st[ic] = {
                        "te8": awp.tile([128, MT, CK], FP8, tag="te8", name="te8", bufs=2),
                        "psden": psD.tile([128, 512], F32, tag="den", name="psden"),
                        "accs": None,
                        "aw": {},
                        "den_done": 0,
                    }
                return st[ic]

            _attn_emitted = set()

            def attn_group(ic, g):
                _attn_emitted.add((ic, g))
                cols = slice(ic * CK, (ic + 1) * CK)
                s = chunk_state(ic)
                cp = cpp.tile([128, SG, 2, CK], FP16, tag="cp", name="cp")
                for js in range(SG):
                    j = g * SG + js
                    msl = slice(j * 128, (j + 1) * 128)
                    ps_r = psA.tile([128, 512], F32, tag="pair", name="ps_r")
                    ps_i = psA.tile([128, 512], F32, tag="pair", name="ps_i")
                    for t in (0, 1):
                        mm(ps_r[:, 0:CK], k16["r"][:, t, msl], q16["r"][:, t, cols], t == 0, False)
                        mm(ps_i[:, 0:CK], k16["r"][:, t, msl], q16["i"][:, t, cols], t == 0, False)
                    for t in (0, 1):
                        mm(ps_r[:, 0:CK], k16["i"][:, t, msl], q16["ni"][:, t, cols], False, t == 1)
                        mm(ps_i[:, 0:CK], k16["i"][:, t, msl], q16["r"][:, t, cols], False, t == 1)
                    if js % 2 == 0:
                        nc.scalar.copy(out=cp[:, js, 0, :], in_=ps_r[:, 0:CK])
                        nc.vector.tensor_copy(out=cp[:, js, 1, :], in_=ps_i[:, 0:CK])
                    else:
                        nc.vector.tensor_copy(out=cp[:, js, 0, :], in_=ps_r[:, 0:CK])
                        nc.scalar.copy(out=cp[:, js, 1, :], in_=ps_i[:, 0:CK])
                sq = smp.tile([128, SG, 2, CK], FP16, tag="sq", name="sq")
                nc.vector.tensor_tensor(out=sq, in0=cp, in1=cp, op=OP.mult)
                m2 = smp.tile([128, SG, CK], FP16, tag="m2", name="m2")
                nc.vector.tensor_tensor(out=m2, in0=sq[:, :, 0, :], in1=sq[:, :, 1, :], op=OP.add)
                lf = smp.tile([128, SG, CK], FP16, tag="lf", name="lf")
                nc.scalar.activation(out=lf, in_=m2, func=AF.Ln, bias=lbias)
                mag = smp.tile([128, SG, CK], FP16, tag="mag", name="mag")
                nc.scalar.activation(out=mag, in_=lf, func=AF.Exp, bias=0.0, scale=0.5)
                u = smp.tile([128, SG, CK], FP16, tag="u", name="u")
                nc.vector.scalar_tensor_tensor(out=u, in0=lf, scalar=-0.5,
                                               in1=mag, op0=OP.mult, op1=OP.add)
                tt = smp.tile([128, SG, CK], FP16, tag="tt", name="tt")
                nc.scalar.activation(out=tt, in_=u, func=AF.Exp, bias=ebias)
                nc.scalar.activation(out=s["te8"][:, g * SG:(g + 1) * SG, :], in_=mag,
                                     func=AF.Exp, bias=ebias)
                aw = awp.tile([128, SG, 2, CK], FP16, tag="aw", name="aw")
                nc.vector.tensor_tensor(out=aw[:, :, 0, :], in0=tt, in1=cp[:, :, 0, :], op=OP.mult)
                nc.vector.tensor_tensor(out=aw[:, :, 1, :], in0=tt, in1=cp[:, :, 1, :], op=OP.mult)
                s["aw"][g] = aw

            def vmm_group(ic, g):
                s = chunk_state(ic)
                if s["accs"] is None:
                    s["accs"] = {(p_, co): psACC.tile([128, CK], F32, tag=f"acc{p_}{co}",
                                                      name=f"acc{p_}{co}")
                                 for p_ in ("r", "i") for co in (0, 1)}
                accs = s["accs"]
                aw = s["aw"].pop(g)
                for js in range(SG):
                    j = g * SG + js
                    st_ = (j == 0)
                    sp_ = (j == MT - 1)
                    for co in (0, 1):
                        co_sl = slice(co * 128, (co + 1) * 128)
                        nc.tensor.matmul(accs[("r", co)], vT["r"][:, j, co_sl], aw[:, js, 0, :], start=st_, stop=False, skip_group_check=True)
                        nc.tensor.matmul(accs[("i", co)], vT["r"][:, j, co_sl], aw[:, js, 1, :], start=st_, stop=False, skip_group_check=True)
                        nc.tensor.matmul(accs[("r", co)], vT["ni"][:, j, co_sl], aw[:, js, 1, :], start=False, stop=sp_, skip_group_check=True)
                        nc.tensor.matmul(accs[("i", co)], vT["i"][:, j, co_sl], aw[:, js, 0, :], start=False, stop=sp_, skip_group_check=True)
                # once every attn group of the chunk is emitted, all te8 is
                # in flight: emit the remaining den matmuls eagerly so the
                # tail's 1/den chain overlaps the final vmm groups
                hi = MT // 2 if (ic, NG - 1) in _attn_emitted else ((g + 1) * SG) // 2
                for jp in range(s["den_done"], hi):
                    nc.tensor.matmul(s["psden"][0:1, 0:CK], ones8[:, :, 0:1],
                                     s["te8"][:, 2 * jp:2 * jp + 2, :],
                                     start=(jp == 0), stop=(jp == MT // 2 - 1),
                                     perf_mode=DRM, skip_group_check=True)
                s["den_done"] = hi

            def tail(ic):
                cols = slice(ic * CK, (ic + 1) * CK)
                s = st.pop(ic)
                psden, accs = s["psden"], s["accs"]
                lden = ckp.tile([1, CK], F32, tag="lden", name="lden")
                nc.scalar.activation(out=lden, in_=psden[0:1, 0:CK], func=AF.Ln, bias=0.0)
                rec16 = ckp.tile([1, CK], FP16, tag="rec16", name="rec16")
                nc.scalar.activation(out=rec16, in_=lden, func=AF.Exp, bias=0.0, scale=-1.0)
                nc.tensor.matmul(psden[:, 0:CK], ones_row[:], rec16[:], start=True, stop=True, skip_group_check=True)
                sden = ckp.tile([128, CK], FP16, tag="sden", name="sden")
                nc.scalar.copy(out=sden, in_=psden[:, 0:CK])

                outsc = {p_: ckp.tile([128, 2, CK], FP16, tag=f"osc{p_}", name=f"osc{p_}")
                         for p_ in ("r", "i")}
                for p_ in ("r", "i"):
                    for co in (0, 1):
                        nc.vector.tensor_tensor(out=outsc[p_][:, co, :], in0=accs[(p_, co)],
                                                in1=sden, op=OP.mult)

                for co in (0, 1):
                    co_sl = slice(co * 128, (co + 1) * 128)
                    ps_pr = psACC.tile([128, CK], F32, tag="accr0", name="ps_pr")
                    ps_pi = psACC.tile([128, CK], F32, tag="acci0", name="ps_pi")
                    for t in (0, 1):
                        mm(ps_pr, wpk["wpr"][:, t, co_sl], outsc["r"][:, t, :], t == 0, False)
                        mm(ps_pi, wpk["wpr"][:, t, co_sl], outsc["i"][:, t, :], t == 0, False)
                    for t in (0, 1):
                        mm(ps_pr, wpk["nwpi"][:, t, co_sl], outsc["i"][:, t, :], False, t == 1)
                        mm(ps_pi, wpk["wpi"][:, t, co_sl], outsc["r"][:, t, :], False, t == 1)
                    for p_, ps_p in (("r", ps_pr), ("i", ps_pi)):
                        # xr32/xi32 carry x + p-bias (host-folded)
                        xres = ckp.tile([128, CK], F32, tag=f"xres{p_}{co}",
                                        name=f"xres{p_}{co}", bufs=1)
                        nc.sync.dma_start(out=xres, in_=d["xr32" if p_ == "r" else "xi32"][co * 128:(co + 1) * 128, cols])
                        nc.vector.scalar_tensor_tensor(out=xres, in0=ps_p, scalar=1.0,
                                                       in1=xres, op0=OP.mult, op1=OP.add)
                        nc.sync.dma_start(out=d["outr" if p_ == "r" else "outi"][co * 128:(co + 1) * 128, cols], in_=xres)

            # global group stream: chunk ic's tail interleaves with chunk
            # ic+1's first attention groups, so the PE never idles on the
            # denominator/outsc/p-proj chain.
            TOT = NCK * NG
            for gg in range(TOT + LAG):
                ic, g = divmod(gg, NG)
                if gg < TOT:
                    with nc.named_scope(f"attn_chunk{ic}"):
                        attn_group(ic, g)
                vg = gg - LAG
                if vg >= 0:
                    vic, vgg = divmod(vg, NG)
                    with nc.named_scope(f"attn_chunk{vic}"):
                        vmm_group(vic, vgg)
                        if vgg == NG - 1:
                            tail(vic)


_CACHE = {}


def _build():
    if "nc" in _CACHE:
        return _CACHE["nc"]
    nc = bacc.Bacc("TRN2", target_bir_lowering=False, debug=False, num_devices=8)
    d = {}
    ins = {
        "x16r": ([128, 2, N], FP16), "x16i": ([128, 2, N], FP16),
        "xr32": ([C, N], F32), "xi32": ([C, N], F32),
        "vecs": ([128, 2, 9], F32),
        "bvr": ([1, C], FP16), "bvi": ([1, C], FP16), "nbvi": ([1, C], FP16),
        "selmat": ([128, 16], F32), "expmat": ([16, 128], F32),
        "ones8": ([128, 2, 16], FP8), "ones_row": ([1, 128], FP16),
    }
    for nm in ("wqr", "wqi", "nwqi", "wkr", "wki", "nwki",
               "wvr", "wvi", "nwvi", "wpr", "wpi", "nwpi"):
        ins[nm] = ([128, 2, 256], FP16)
    for nm, (sh, dt_) in ins.items():
        d[nm] = nc.dram_tensor(nm, list(sh), dt_, kind="ExternalInput").ap()
    for nm in ("outr", "outi"):
        d[nm] = nc.dram_tensor(nm, [C, NQ], F32, kind="ExternalOutput").ap()
    with tile.TileContext(nc) as tc:
        _emit(nc, tc, d)
    import concourse.bacc as _bacc_mod
    _orig_tables = _bacc_mod.get_activation_tables

    def _tables_ln_exp_combined(arch):
        # empty out every other set so ALL activation funcs resolve to the one
        # set holding ln+exp -> exactly one ACT_TABLE_LOAD in the whole kernel
        tabs = _orig_tables(arch)
        return {
            name: (fns if name == "natural_log_exp_and_others" else set())
            for name, fns in tabs.items()
        }

    _bacc_mod.get_activation_tables = _tables_ln_exp_combined
    try:
        nc.compile()
    finally:
        _bacc_mod.get_activation_tables = _orig_tables
    _CACHE["nc"] = nc
    return nc


def _pk16(wT):
    """[C, 256] (c, o) weight -> pack [128, 2, 256] fp16."""
    return np.ascontiguousarray(
        np.asarray(wT, np.float32).reshape(2, 128, 256).transpose(1, 0, 2)
    ).astype(np.float16)


def kernel(x_ri, gn_gamma, gn_beta, qw, qb, kw, kb, vw, vb, pw, pb):
    x_ri = np.asarray(x_ri, np.float32)
    f = lambda a: np.ascontiguousarray(np.asarray(a, np.float32))
    qw, qb = np.asarray(qw, np.float32), np.asarray(qb, np.float32)
    kw, kb = np.asarray(kw, np.float32), np.asarray(kb, np.float32)
    vw, vb = np.asarray(vw, np.float32), np.asarray(vb, np.float32)
    pw, pb = np.asarray(pw, np.float32), np.asarray(pb, np.float32)
    s4 = float(C) ** -0.25

    common = {
        "wqr": _pk16(qw[0].T * s4), "wqi": _pk16(qw[1].T * s4), "nwqi": _pk16(-qw[1].T * s4),
        "wkr": _pk16(kw[0].T * s4), "wki": _pk16(kw[1].T * s4), "nwki": _pk16(-kw[1].T * s4),
        "wvr": _pk16(vw[0].T), "wvi": _pk16(vw[1].T), "nwvi": _pk16(-vw[1].T),
        "wpr": _pk16(pw[0].T), "wpi": _pk16(pw[1].T), "nwpi": _pk16(-pw[1].T),
        "bvr": np.ascontiguousarray(vb[0].reshape(1, C).astype(np.float16)),
        "bvi": np.ascontiguousarray(vb[1].reshape(1, C).astype(np.float16)),
        "nbvi": np.ascontiguousarray((-vb[1]).reshape(1, C).astype(np.float16)),
        "selmat": np.eye(16, dtype=np.float32)[np.arange(128) // 8],
        "expmat": np.ascontiguousarray(np.eye(16, dtype=np.float32)[np.arange(128) // 8].T),
        "ones_row": np.ones((1, 128), np.float16),
    }
    ones8 = np.zeros((128, 2, 16), np.float32)
    ones8[:, :, 0] = 1.0
    common["ones8"] = np.ascontiguousarray(ones8.astype(ml_dtypes.float8_e4m3))
    # all [C]-shaped vectors in one DMA: vecs[p, t, idx], channel c = 128t+p
    vv = np.stack([qb[0] * s4, qb[1] * s4, -qb[1] * s4, kb[0] * s4, kb[1] * s4,
                   gn_gamma[0::2], gn_gamma[1::2], gn_beta[0::2], gn_beta[1::2]],
                  axis=1).astype(np.float32)          # [C, 9]
    common["vecs"] = np.ascontiguousarray(vv.reshape(2, 128, 9).transpose(1, 0, 2))

    xr = np.ascontiguousarray(x_ri[..., 0].reshape(B, C, N))
    xi = np.ascontiguousarray(x_ri[..., 1].reshape(B, C, N))

    def pack16(a):  # [C, N] -> [128, 2, N] fp16
        return np.ascontiguousarray(a.reshape(2, 128, N).transpose(1, 0, 2).astype(np.float16))

    pbr_col = pb[0].reshape(C, 1)
    pbi_col = pb[1].reshape(C, 1)
    in_maps = []
    for core in range(8):
        b, half = core // 2, core % 2
        q0 = half * NQ
        xrr = np.ascontiguousarray(np.roll(xr[b], -q0, axis=1))
        xir = np.ascontiguousarray(np.roll(xi[b], -q0, axis=1))
        in_maps.append({
            **common,
            "x16r": pack16(xrr), "x16i": pack16(xir),
            "xr32": np.ascontiguousarray(xrr + pbr_col),
            "xi32": np.ascontiguousarray(xir + pbi_col),
        })

    nc = _build()
    trace = os.environ.get("BASS_KERNEL_TRACE") == "1"
    res = run_bass_kernel_spmd(nc, in_maps, core_ids=list(range(8)), trace=trace)
    kernel._last_result = res

    out = np.empty((B, C, N), np.complex64)
    for core in range(8):
        b, half = core // 2, core % 2
        q0 = half * NQ
        rr = res.results[core]
        out[b, :, q0:q0 + NQ] = rr["outr"] + 1j * rr["outi"]
    return out.reshape(B, C, HH, WW)


kernel._last_result = None


# revision 5
# speedup vs baseline: 1.0162x; 1.0035x over previous
"""Trainium2 Bass kernel for a complex-valued attention block — v3.

Reference computation (per batch sample, complex64):
  h = ComplexGroupNorm(x); q,k,v = 1x1 complex convs of h
  attn = (q^T k) * C^-0.5; aw = magnitude-softmax(attn)
  out = v @ aw^T; y = x + 1x1conv_p(out)

Sharding: 8 cores = 4 batches x 2 query-halves (x rolled so each core's
query block is columns [0, 1152)).

v3 (vs the two-stage v1):
  - one-pass softmax per group of SG=3 key tiles with LAGGED v-matmul
    emission: the PE queue runs group g+2's score matmuls while group g's
    ln/exp chain completes, so the tensor engine never waits on softmax.
  - all matmuls fp16 (fp8 scores fail: the peaked softmax amplifies
    quantization; measured 3e-2 rel err) except the denominator, which is
    an fp8 DoubleRow ones-matmul over te pairs (te8 errs only ~2e-3).
  - EXP_SHIFT 2^-8 keeps te8 under the fp8e4 max (peak |attn| is 10.9).
  - e^mag via a 4th ACT pass writing fp8 directly; 1/den = exp(-ln(den))
    on ACT straight from PSUM; negated vT / negated q_i copies instead of
    per-chunk negations; all softmax intermediates fp16 (lbias=1e-8 keeps
    them in range); p-bias folded into the residual on the host.
"""

import os
import ml_dtypes
import numpy as np

import concourse.bacc as bacc
import concourse.bass as bass
import concourse.tile as tile
from concourse import mybir
from concourse.bass_utils import run_bass_kernel_spmd

B, C, HH, WW = 4, 256, 48, 48
N = HH * WW            # 2304 keys
NQ = N // 2            # 1152 queries per core
CK = 384               # query-chunk width
NCK = NQ // CK         # 3 chunks
MT = N // 128          # 18 key tiles
SG = 3
NG = MT // SG          # 6 softmax groups per chunk
LAG = 2                # groups between score emission and v-matmul emission
GN_EPS = 1e-6

F32 = mybir.dt.float32
FP16 = mybir.dt.float16
FP8 = mybir.dt.float8e4
EXP_SHIFT = -5.545177444479562  # -8*ln2; cancels via the denominator
LBIAS = 1e-8

AF = mybir.ActivationFunctionType
OP = mybir.AluOpType
DRM = mybir.MatmulPerfMode.DoubleRow


def _emit(nc, tc, d):
    def pool(name, bufs, space="SBUF"):
        return tc.tile_pool(name=name, bufs=bufs, space=space)

    def mm(out, lhsT, rhs, start, stop):
        nc.tensor.matmul(out, lhsT, rhs, start=start, stop=stop,
                         skip_group_check=True)

    with pool("const", 1) as const, pool("persist", 1) as pers:

        # ---- transposed fp8 x first: GN stats are the critical path at kernel
        # start, and they run as matmuls on the otherwise-idle tensor engine
        # (spatial dim on partitions -> channel sums are N=1 matmuls) ----
        xT8 = {p_: pers.tile([128, MT, 256], FP8, tag=f"xT8{p_}", name=f"xT8{p_}")
               for p_ in ("r", "i")}
        for piece in range(3):
            jsl = slice(piece * 6, (piece + 1) * 6)
            for p_ in ("r", "i"):
                nc.sync.dma_start(out=xT8[p_][:, jsl, :],
                                  in_=d["xT8r" if p_ == "r" else "xT8i"][:, jsl, :])
        x16 = {p_: pers.tile([128, 2, N], FP16, tag=f"x16{p_}", name=f"x16{p_}")
               for p_ in ("r", "i")}
        for p_ in ("r", "i"):
            for t in (0, 1):
                nc.sync.dma_start(out=x16[p_][:, t, :],
                                  in_=d["x16r" if p_ == "r" else "x16i"][:, t, :])

        # ---- GN-chain consts, then ONE packed vecs DMA (each dma_start costs
        # ~600ns of sync-queue issue time; 26 tiny vector DMAs would delay the
        # weight packs by >10us) ----
        selmat = const.tile([128, 16], F32, tag="selmat", name="selmat")
        nc.sync.dma_start(out=selmat, in_=d["selmat"])
        expmat = const.tile([16, 128], F32, tag="expmat", name="expmat")
        nc.sync.dma_start(out=expmat, in_=d["expmat"])
        vecs = const.tile([128, 2, 9], F32, tag="vecs", name="vecs")
        nc.sync.dma_start(out=vecs, in_=d["vecs"])
        VIDX = {"qbr": 0, "qbi": 1, "nqbi": 2, "kbr": 3, "kbi": 4,
                "gam_r": 5, "gam_i": 6, "bet_r": 7, "bet_i": 8}
        gam = {(p_, t): vecs[:, t, VIDX[f"gam_{p_}"]:VIDX[f"gam_{p_}"] + 1]
               for p_ in ("r", "i") for t in (0, 1)}
        bet = {(p_, t): vecs[:, t, VIDX[f"bet_{p_}"]:VIDX[f"bet_{p_}"] + 1]
               for p_ in ("r", "i") for t in (0, 1)}
        qkb = {(nm, co): vecs[:, co, VIDX[nm]:VIDX[nm] + 1]
               for nm in ("qbr", "qbi", "nqbi", "kbr", "kbi") for co in (0, 1)}

        # ---- weight packs (needed once projections start) ----
        wpk = {}
        for nm in ("wqr", "wqi", "nwqi", "wkr", "wki", "nwki",
                   "wvr", "wvi", "nwvi", "wpr", "wpi", "nwpi"):
            t = const.tile([128, 2, 256], FP16, tag=nm, name=nm)
            nc.sync.dma_start(out=t, in_=d[nm])
            wpk[nm] = t

        bvrow = {}
        for nm in ("bvr", "bvi", "nbvi"):
            t = const.tile([1, 256], FP16, tag=nm, name=nm)
            nc.sync.dma_start(out=t, in_=d[nm])
            bvrow[nm] = t
        ones8 = const.tile([128, 2, 16], FP8, tag="ones8", name="ones8")
        nc.sync.dma_start(out=ones8, in_=d["ones8"])
        ones_row = const.tile([1, 128], FP16, tag="ones_row", name="ones_row")
        nc.sync.dma_start(out=ones_row, in_=d["ones_row"])
        ebias = const.tile([128, 1], F32, tag="ebias", name="ebias")
        nc.vector.memset(ebias, EXP_SHIFT)
        lbias = const.tile([128, 1], F32, tag="lbias", name="lbias")
        nc.vector.memset(lbias, LBIAS)

        # ---- persistent activations ----
        h16 = {p_: pers.tile([128, 2, N], FP16, tag=f"h16{p_}", name=f"h16{p_}")
               for p_ in ("r", "i")}
        q16 = {nm: pers.tile([128, 2, NQ], FP16, tag=f"q16{nm}", name=f"q16{nm}")
               for nm in ("r", "i", "ni")}
        k16 = {nm: pers.tile([128, 2, N], FP16, tag=f"k16{nm}", name=f"k16{nm}")
               for nm in ("r", "i")}
        vT = {nm: pers.tile([128, MT, 256], FP16, tag=f"vT{nm}", name=f"vT{nm}")
              for nm in ("r", "i", "ni")}

        # ================= phase 1: GroupNorm -> h16 packs =================
        with pool("gnw", 2) as gnw, pool("ps_small", 1, "PSUM") as ps_small, \
             pool("psst", 4, "PSUM") as psst, nc.named_scope("groupnorm"):
            ones_c16 = const.tile([128, 1], FP16, tag="ones_c16", name="ones_c16")
            nc.vector.memset(ones_c16, 1.0)
            ones_c8 = const.tile([128, 1], FP8, tag="ones_c8", name="ones_c8")
            nc.vector.memset(ones_c8, 1.0)
            # squares of the transposed copy on the (idle) ACT engine
            sq16 = {p_: gnw.tile([128, MT, 256], FP16, tag=f"sq16{p_}",
                                 name=f"sq16{p_}", bufs=1) for p_ in ("r", "i")}
            for piece in range(3):
                jsl = slice(piece * 6, (piece + 1) * 6)
                for p_ in ("r", "i"):
                    nc.scalar.activation(out=sq16[p_][:, jsl, :], in_=xT8[p_][:, jsl, :],
                                         func=AF.Square, bias=0.0, scale=1.0)
            # per-channel sums / squared sums via N=1 matmuls, accumulated
            # over the 18 spatial tiles; srhs [128, 8] = [t, part, (mean, E2)]
            srhs = gnw.tile([128, 8], F32, tag="srhs", name="srhs", bufs=1)
            for t in (0, 1):
                co_sl = slice(t * 128, (t + 1) * 128)
                for ci, p_ in enumerate(("r", "i")):
                    o = 4 * t + 2 * ci
                    psS = psst.tile([128, 1], F32, tag="psS", name="psS")
                    for jt in range(MT):
                        mm(psS, xT8[p_][:, jt, co_sl], ones_c8[:], jt == 0, jt == MT - 1)
                    nc.vector.tensor_scalar_mul(srhs[:, o:o + 1], psS, 1.0 / float(N))
                    psQ = psst.tile([128, 1], F32, tag="psS", name="psQ")
                    for jt in range(MT):
                        mm(psQ, sq16[p_][:, jt, co_sl], ones_c16[:], jt == 0, jt == MT - 1)
                    nc.vector.tensor_scalar_mul(srhs[:, o + 1:o + 2], psQ, 1.0 / float(N))
            psg = ps_small.tile([16, 8], F32, tag="psg", name="psg")
            nc.tensor.matmul(psg, selmat, srhs, start=True, stop=True)
            gsum = gnw.tile([16, 8], F32, tag="gsum", name="gsum", bufs=1)
            nc.scalar.copy(out=gsum, in_=psg)
            gv = gsum[:].rearrange("p (t c) -> p t c", c=4)
            # st4 [16, (mu_t0, rstd_t0, mu_t1, rstd_t1)]
            st4 = gnw.tile([16, 2, 2], F32, tag="st4", name="st4", bufs=1)
            mu = st4[:, :, 0:1]
            nc.vector.tensor_tensor(out=mu, in0=gv[:, :, 0:1], in1=gv[:, :, 2:3], op=OP.add)
            nc.vector.tensor_scalar_mul(mu, mu, 1.0 / 16.0)
            ex2 = gnw.tile([16, 2], F32, tag="gex2", name="gex2")
            nc.vector.tensor_tensor(out=ex2, in0=gv[:, :, 1:2], in1=gv[:, :, 3:4], op=OP.add)
            nc.vector.tensor_scalar_mul(ex2, ex2, 1.0 / 16.0)
            mu2 = gnw.tile([16, 2], F32, tag="gmu2", name="gmu2")
            nc.vector.tensor_tensor(out=mu2, in0=mu[:, :, 0], in1=mu[:, :, 0], op=OP.mult)
            var = gnw.tile([16, 2], F32, tag="gvar", name="gvar")
            nc.vector.tensor_tensor(out=var, in0=ex2, in1=mu2, op=OP.subtract)
            epst = gnw.tile([16, 1], F32, tag="geps", name="geps", bufs=1)
            nc.vector.memset(epst, GN_EPS)
            lnv = gnw.tile([16, 2], F32, tag="glnv", name="glnv")
            nc.scalar.activation(out=lnv, in_=var, func=AF.Ln, bias=epst, scale=1.0)
            nc.scalar.activation(out=st4[:, :, 1:2], in_=lnv, func=AF.Exp, bias=0.0, scale=-0.5)
            psr = ps_small.tile([128, 4], F32, tag="psr", name="psr")
            nc.tensor.matmul(psr, expmat, st4, start=True, stop=True)
            gst = gnw.tile([128, 2, 2], F32, tag="gst", name="gst", bufs=1)
            nc.scalar.copy(out=gst, in_=psr)
            for p_ in ("r", "i"):
                for t in (0, 1):
                    sc = gnw.tile([128, 1], F32, tag=f"sc{p_}{t}", name=f"sc{p_}{t}", bufs=1)
                    nc.vector.tensor_tensor(out=sc, in0=gst[:, t, 1:2], in1=gam[(p_, t)], op=OP.mult)
                    bi = gnw.tile([128, 1], F32, tag=f"bi{p_}{t}", name=f"bi{p_}{t}", bufs=1)
                    nc.vector.tensor_tensor(out=bi, in0=gst[:, t, 0:1], in1=sc, op=OP.mult)
                    nc.vector.scalar_tensor_tensor(out=bi, in0=bi, scalar=-1.0,
                                                   in1=bet[(p_, t)], op0=OP.mult, op1=OP.add)
                    # apply on DVE (fp16 tensor_scalar runs 4x; ACT pays 2.2us/tile)
                    nc.vector.tensor_scalar(out=h16[p_][:, t, :], in0=x16[p_][:, t, :],
                                            scalar1=sc, scalar2=bi, op0=OP.mult, op1=OP.add)

        # ============ phase 2: q/k/vT projections (fp16) ============
        with pool("ps_proj", 4, "PSUM") as psp, nc.named_scope("qkv"):
            for ic in range(NCK):
                cols = slice(ic * CK, (ic + 1) * CK)
                for co in (0, 1):
                    co_sl = slice(co * 128, (co + 1) * 128)
                    ps_r = psp.tile([128, CK], F32, tag="pp", name="ps_qr")
                    ps_i = psp.tile([128, CK], F32, tag="pp", name="ps_qi")
                    for t in (0, 1):
                        mm(ps_r, wpk["wqr"][:, t, co_sl], h16["r"][:, t, cols], t == 0, False)
                        mm(ps_i, wpk["wqr"][:, t, co_sl], h16["i"][:, t, cols], t == 0, False)
                    for t in (0, 1):
                        mm(ps_r, wpk["nwqi"][:, t, co_sl], h16["i"][:, t, cols], False, t == 1)
                        mm(ps_i, wpk["wqi"][:, t, co_sl], h16["r"][:, t, cols], False, t == 1)
                    nc.scalar.activation(out=q16["r"][:, co, cols], in_=ps_r,
                                         func=AF.Identity, bias=qkb[("qbr", co)], scale=1.0)
                    nc.scalar.activation(out=q16["i"][:, co, cols], in_=ps_i,
                                         func=AF.Identity, bias=qkb[("qbi", co)], scale=1.0)
                    nc.vector.tensor_scalar(out=q16["ni"][:, co, cols], in0=ps_i,
                                            scalar1=-1.0, scalar2=qkb[("nqbi", co)],
                                            op0=OP.mult, op1=OP.add)
            for ic in range(N // CK):
                cols = slice(ic * CK, (ic + 1) * CK)
                for co in (0, 1):
                    co_sl = slice(co * 128, (co + 1) * 128)
                    ps_r = psp.tile([128, CK], F32, tag="pp", name="ps_kr")
                    ps_i = psp.tile([128, CK], F32, tag="pp", name="ps_ki")
                    for t in (0, 1):
                        mm(ps_r, wpk["wkr"][:, t, co_sl], h16["r"][:, t, cols], t == 0, False)
                        mm(ps_i, wpk["wkr"][:, t, co_sl], h16["i"][:, t, cols], t == 0, False)
                    for t in (0, 1):
                        mm(ps_r, wpk["nwki"][:, t, co_sl], h16["i"][:, t, cols], False, t == 1)
                        mm(ps_i, wpk["wki"][:, t, co_sl], h16["r"][:, t, cols], False, t == 1)
                    nc.vector.tensor_scalar_add(k16["r"][:, co, cols], ps_r, qkb[("kbr", co)])
                    nc.vector.tensor_scalar_add(k16["i"][:, co, cols], ps_i, qkb[("kbi", co)])
            # bv broadcast emitted here (not at kernel start) so the tiny
            # matmuls don't block the GN aggregation matmul in the PE queue
            bv_bc = {}
            for nm in ("bvr", "bvi", "nbvi"):
                psb = psp.tile([128, 256], F32, tag="pp", name="psb")
                nc.tensor.matmul(psb, ones_row[:], bvrow[nm][:], start=True, stop=True)
                t = const.tile([128, 256], F32, tag=nm + "_bc", name=nm + "_bc")
                nc.scalar.copy(out=t, in_=psb)
                bv_bc[nm] = t
            with nc.named_scope("vproj"):
                for j in range(MT):
                    msl = slice(j * 128, (j + 1) * 128)
                    ps_vr = psp.tile([128, 256], F32, tag="pp", name="ps_vr")
                    ps_vi = psp.tile([128, 256], F32, tag="pp", name="ps_vi")
                    for t in (0, 1):
                        mm(ps_vr, h16["r"][:, t, msl], wpk["wvr"][:, t, :], t == 0, False)
                        mm(ps_vi, h16["r"][:, t, msl], wpk["wvi"][:, t, :], t == 0, False)
                    for t in (0, 1):
                        mm(ps_vr, h16["i"][:, t, msl], wpk["nwvi"][:, t, :], False, t == 1)
                        mm(ps_vi, h16["i"][:, t, msl], wpk["wvr"][:, t, :], False, t == 1)
                    nc.vector.tensor_tensor(out=vT["r"][:, j, :], in0=ps_vr, in1=bv_bc["bvr"], op=OP.add)
                    nc.vector.tensor_tensor(out=vT["i"][:, j, :], in0=ps_vi, in1=bv_bc["bvi"], op=OP.add)
                    nc.vector.scalar_tensor_tensor(out=vT["ni"][:, j, :], in0=ps_vi, scalar=-1.0,
                                                   in1=bv_bc["nbvi"], op0=OP.mult, op1=OP.add)

        # ============ phase 3: attention (one pass, lagged v-matmul) ============
        with pool("psA", 3, "PSUM") as psA, pool("psACC", 1, "PSUM") as psACC, \
             pool("psD", 1, "PSUM") as psD, \
             pool("cpp", 2) as cpp, pool("smp", 2) as smp, pool("awp", LAG + 1) as awp, \
             pool("ckp", 2) as ckp:
            # per-chunk state, created lazily as the global group stream reaches it
            st = {}

            def chunk_state(ic):
                if ic not in st:
                    st[ic] = {
                        "te8": awp.tile([128, MT, CK], FP8, tag="te8", name="te8", bufs=2),
                        "psden": psD.tile([128, 512], F32, tag="den", name="psden"),
                        "accs": None,
                        "aw": {},
                        "den_done": 0,
                    }
                return st[ic]

            _attn_emitted = set()

            def attn_group(ic, g):
                _attn_emitted.add((ic, g))
                cols = slice(ic * CK, (ic + 1) * CK)
                s = chunk_state(ic)
                cp = cpp.tile([128, SG, 2, CK], FP16, tag="cp", name="cp")
                for js in range(SG):
                    j = g * SG + js
                    msl = slice(j * 128, (j + 1) * 128)
                    ps_r = psA.tile([128, 512], F32, tag="pair", name="ps_r")
                    ps_i = psA.tile([128, 512], F32, tag="pair", name="ps_i")
                    for t in (0, 1):
                        mm(ps_r[:, 0:CK], k16["r"][:, t, msl], q16["r"][:, t, cols], t == 0, False)
                        mm(ps_i[:, 0:CK], k16["r"][:, t, msl], q16["i"][:, t, cols], t == 0, False)
                    for t in (0, 1):
                        mm(ps_r[:, 0:CK], k16["i"][:, t, msl], q16["ni"][:, t, cols], False, t == 1)
                        mm(ps_i[:, 0:CK], k16["i"][:, t, msl], q16["r"][:, t, cols], False, t == 1)
                    if js % 2 == 0:
                        nc.scalar.copy(out=cp[:, js, 0, :], in_=ps_r[:, 0:CK])
                        nc.vector.tensor_copy(out=cp[:, js, 1, :], in_=ps_i[:, 0:CK])
                    else:
                        nc.vector.tensor_copy(out=cp[:, js, 0, :], in_=ps_r[:, 0:CK])
                        nc.scalar.copy(out=cp[:, js, 1, :], in_=ps_i[:, 0:CK])
                sq = smp.tile([128, SG, 2, CK], FP16, tag="sq", name="sq")
                nc.vector.tensor_tensor(out=sq, in0=cp, in1=cp, op=OP.mult)
                m2 = smp.tile([128, SG, CK], FP16, tag="m2", name="m2")
                nc.vector.tensor_tensor(out=m2, in0=sq[:, :, 0, :], in1=sq[:, :, 1, :], op=OP.add)
                lf = smp.tile([128, SG, CK], FP16, tag="lf", name="lf")
                nc.scalar.activation(out=lf, in_=m2, func=AF.Ln, bias=lbias)
                mag = smp.tile([128, SG, CK], FP16, tag="mag", name="mag")
                nc.scalar.activation(out=mag, in_=lf, func=AF.Exp, bias=0.0, scale=0.5)
                u = smp.tile([128, SG, CK], FP16, tag="u", name="u")
                nc.vector.scalar_tensor_tensor(out=u, in0=lf, scalar=-0.5,
                                               in1=mag, op0=OP.mult, op1=OP.add)
                tt = smp.tile([128, SG, CK], FP16, tag="tt", name="tt")
                nc.scalar.activation(out=tt, in_=u, func=AF.Exp, bias=ebias)
                nc.scalar.activation(out=s["te8"][:, g * SG:(g + 1) * SG, :], in_=mag,
                                     func=AF.Exp, bias=ebias)
                aw = awp.tile([128, SG, 2, CK], FP16, tag="aw", name="aw")
                nc.vector.tensor_tensor(out=aw[:, :, 0, :], in0=tt, in1=cp[:, :, 0, :], op=OP.mult)
                nc.vector.tensor_tensor(out=aw[:, :, 1, :], in0=tt, in1=cp[:, :, 1, :], op=OP.mult)
                s["aw"][g] = aw

            def vmm_group(ic, g):
                s = chunk_state(ic)
                if s["accs"] is None:
                    s["accs"] = {(p_, co): psACC.tile([128, CK], F32, tag=f"acc{p_}{co}",
                                                      name=f"acc{p_}{co}")
                                 for p_ in ("r", "i") for co in (0, 1)}
                accs = s["accs"]
                aw = s["aw"].pop(g)
                for js in range(SG):
                    j = g * SG + js
                    st_ = (j == 0)
                    sp_ = (j == MT - 1)
                    for co in (0, 1):
                        co_sl = slice(co * 128, (co + 1) * 128)
                        nc.tensor.matmul(accs[("r", co)], vT["r"][:, j, co_sl], aw[:, js, 0, :], start=st_, stop=False, skip_group_check=True)
                        nc.tensor.matmul(accs[("i", co)], vT["r"][:, j, co_sl], aw[:, js, 1, :], start=st_, stop=False, skip_group_check=True)
                        nc.tensor.matmul(accs[("r", co)], vT["ni"][:, j, co_sl], aw[:, js, 1, :], start=False, stop=sp_, skip_group_check=True)
                        nc.tensor.matmul(accs[("i", co)], vT["i"][:, j, co_sl], aw[:, js, 0, :], start=False, stop=sp_, skip_group_check=True)
                # once every attn group of the chunk is emitted, all te8 is
                # in flight: emit the remaining den matmuls eagerly so the
                # tail's 1/den chain overlaps the final vmm groups
                hi = MT // 2 if (ic, NG - 1) in _attn_emitted else ((g + 1) * SG) // 2
                for jp in range(s["den_done"], hi):
                    nc.tensor.matmul(s["psden"][0:1, 0:CK], ones8[:, :, 0:1],
                                     s["te8"][:, 2 * jp:2 * jp + 2, :],
                                     start=(jp == 0), stop=(jp == MT // 2 - 1),
                                     perf_mode=DRM, skip_group_check=True)
                s["den_done"] = hi

            def tail(ic):
                cols = slice(ic * CK, (ic + 1) * CK)
                s = st.pop(ic)
                psden, accs = s["psden"], s["accs"]
                lden = ckp.tile([1, CK], F32, tag="lden", name="lden")
                nc.scalar.activation(out=lden, in_=psden[0:1, 0:CK], func=AF.Ln, bias=0.0)
                rec16 = ckp.tile([1, CK], FP16, tag="rec16", name="rec16")
                nc.scalar.activation(out=rec16, in_=lden, func=AF.Exp, bias=0.0, scale=-1.0)
                nc.tensor.matmul(psden[:, 0:CK], ones_row[:], rec16[:], start=True, stop=True, skip_group_check=True)
                sden = ckp.tile([128, CK], FP16, tag="sden", name="sden")
                nc.scalar.copy(out=sden, in_=psden[:, 0:CK])

                outsc = {p_: ckp.tile([128, 2, CK], FP16, tag=f"osc{p_}", name=f"osc{p_}")
                         for p_ in ("r", "i")}
                for p_ in ("r", "i"):
                    for co in (0, 1):
                        nc.vector.tensor_tensor(out=outsc[p_][:, co, :], in0=accs[(p_, co)],
                                                in1=sden, op=OP.mult)

                for co in (0, 1):
                    co_sl = slice(co * 128, (co + 1) * 128)
                    ps_pr = psACC.tile([128, CK], F32, tag="accr0", name="ps_pr")
                    ps_pi = psACC.tile([128, CK], F32, tag="acci0", name="ps_pi")
                    for t in (0, 1):
                        mm(ps_pr, wpk["wpr"][:, t, co_sl], outsc["r"][:, t, :], t == 0, False)
                        mm(ps_pi, wpk["wpr"][:, t, co_sl], outsc["i"][:, t, :], t == 0, False)
                    for t in (0, 1):
                        mm(ps_pr, wpk["nwpi"][:, t, co_sl], outsc["i"][:, t, :], False, t == 1)
                        mm(ps_pi, wpk["wpi"][:, t, co_sl], outsc["r"][:, t, :], False, t == 1)
                    for p_, ps_p in (("r", ps_pr), ("i", ps_pi)):
                        # xr32/xi32 carry x + p-bias (host-folded)
                        xres = ckp.tile([128, CK], F32, tag=f"xres{p_}{co}",
                                        name=f"xres{p_}{co}", bufs=1)
                        nc.sync.dma_start(out=xres, in_=d["xr32" if p_ == "r" else "xi32"][co * 128:(co + 1) * 128, cols])
                        nc.vector.scalar_tensor_tensor(out=xres, in0=ps_p, scalar=1.0,
                                                       in1=xres, op0=OP.mult, op1=OP.add)
                        nc.sync.dma_start(out=d["outr" if p_ == "r" else "outi"][co * 128:(co + 1) * 128, cols], in_=xres)

            # global group stream: chunk ic's tail interleaves with chunk
            # ic+1's first attention groups, so the PE never idles on the
            # denominator/outsc/p-proj chain.
            TOT = NCK * NG
            for gg in range(TOT + LAG):
                ic, g = divmod(gg, NG)
                if gg < TOT:
                    with nc.named_scope(f"attn_chunk{ic}"):
                        attn_group(ic, g)
                vg = gg - LAG
                if vg >= 0:
                    vic, vgg = divmod(vg, NG)
                    with nc.named_scope(f"attn_chunk{vic}"):
                        vmm_group(vic, vgg)
                        if vgg == NG - 1:
                            tail(vic)


_CACHE = {}


def _build():
    if "nc" in _CACHE:
        return _CACHE["nc"]
    nc = bacc.Bacc("TRN2", target_bir_lowering=False, debug=False, num_devices=8)
    d = {}
    ins = {
        "x16r": ([128, 2, N], FP16), "x16i": ([128, 2, N], FP16),
        "xT8r": ([128, MT, 256], FP8), "xT8i": ([128, MT, 256], FP8),
        "xr32": ([C, N], F32), "xi32": ([C, N], F32),
        "vecs": ([128, 2, 9], F32),
        "bvr": ([1, C], FP16), "bvi": ([1, C], FP16), "nbvi": ([1, C], FP16),
        "selmat": ([128, 16], F32), "expmat": ([16, 128], F32),
        "ones8": ([128, 2, 16], FP8), "ones_row": ([1, 128], FP16),
    }
    for nm in ("wqr", "wqi", "nwqi", "wkr", "wki", "nwki",
               "wvr", "wvi", "nwvi", "wpr", "wpi", "nwpi"):
        ins[nm] = ([128, 2, 256], FP16)
    for nm, (sh, dt_) in ins.items():
        d[nm] = nc.dram_tensor(nm, list(sh), dt_, kind="ExternalInput").ap()
    for nm in ("outr", "outi"):
        d[nm] = nc.dram_tensor(nm, [C, NQ], F32, kind="ExternalOutput").ap()
    with tile.TileContext(nc) as tc:
        _emit(nc, tc, d)
    import concourse.bacc as _bacc_mod
    _orig_tables = _bacc_mod.get_activation_tables

    def _tables_ln_exp_combined(arch):
        # empty out every other set so ALL activation funcs resolve to the one
        # set holding ln+exp -> exactly one ACT_TABLE_LOAD in the whole kernel
        tabs = _orig_tables(arch)
        return {
            name: (fns if name == "natural_log_exp_and_others" else set())
            for name, fns in tabs.items()
        }

    _bacc_mod.get_activation_tables = _tables_ln_exp_combined
    try:
        nc.compile()
    finally:
        _bacc_mod.get_activation_tables = _orig_tables
    _CACHE["nc"] = nc
    return nc


def _pk16(wT):
    """[C, 256] (c, o) weight -> pack [128, 2, 256] fp16."""
    return np.ascontiguousarray(
        np.asarray(wT, np.float32).reshape(2, 128, 256).transpose(1, 0, 2)
    ).astype(np.float16)


def kernel(x_ri, gn_gamma, gn_beta, qw, qb, kw, kb, vw, vb, pw, pb):
    x_ri = np.asarray(x_ri, np.float32)
    f = lambda a: np.ascontiguousarray(np.asarray(a, np.float32))
    qw, qb = np.asarray(qw, np.float32), np.asarray(qb, np.float32)
    kw, kb = np.asarray(kw, np.float32), np.asarray(kb, np.float32)
    vw, vb = np.asarray(vw, np.float32), np.asarray(vb, np.float32)
    pw, pb = np.asarray(pw, np.float32), np.asarray(pb, np.float32)
    s4 = float(C) ** -0.25

    common = {
        "wqr": _pk16(qw[0].T * s4), "wqi": _pk16(qw[1].T * s4), "nwqi": _pk16(-qw[1].T * s4),
        "wkr": _pk16(kw[0].T * s4), "wki": _pk16(kw[1].T * s4), "nwki": _pk16(-kw[1].T * s4),
        "wvr": _pk16(vw[0].T), "wvi": _pk16(vw[1].T), "nwvi": _pk16(-vw[1].T),
        "wpr": _pk16(pw[0].T), "wpi": _pk16(pw[1].T), "nwpi": _pk16(-pw[1].T),
        "bvr": np.ascontiguousarray(vb[0].reshape(1, C).astype(np.float16)),
        "bvi": np.ascontiguousarray(vb[1].reshape(1, C).astype(np.float16)),
        "nbvi": np.ascontiguousarray((-vb[1]).reshape(1, C).astype(np.float16)),
        "selmat": np.eye(16, dtype=np.float32)[np.arange(128) // 8],
        "expmat": np.ascontiguousarray(np.eye(16, dtype=np.float32)[np.arange(128) // 8].T),
        "ones_row": np.ones((1, 128), np.float16),
    }
    ones8 = np.zeros((128, 2, 16), np.float32)
    ones8[:, :, 0] = 1.0
    common["ones8"] = np.ascontiguousarray(ones8.astype(ml_dtypes.float8_e4m3))
    # all [C]-shaped vectors in one DMA: vecs[p, t, idx], channel c = 128t+p
    vv = np.stack([qb[0] * s4, qb[1] * s4, -qb[1] * s4, kb[0] * s4, kb[1] * s4,
                   gn_gamma[0::2], gn_gamma[1::2], gn_beta[0::2], gn_beta[1::2]],
                  axis=1).astype(np.float32)          # [C, 9]
    common["vecs"] = np.ascontiguousarray(vv.reshape(2, 128, 9).transpose(1, 0, 2))

    xr = np.ascontiguousarray(x_ri[..., 0].reshape(B, C, N))
    xi = np.ascontiguousarray(x_ri[..., 1].reshape(B, C, N))

    def pack16(a):  # [C, N] -> [128, 2, N] fp16
        return np.ascontiguousarray(a.reshape(2, 128, N).transpose(1, 0, 2).astype(np.float16))

    def packT8(a):  # [C, N] -> [128, MT, 256] fp8 (spatial n = 128*jt + p)
        return np.ascontiguousarray(
            a.T.reshape(MT, 128, C).transpose(1, 0, 2)
        ).astype(ml_dtypes.float8_e4m3)

    pbr_col = pb[0].reshape(C, 1)
    pbi_col = pb[1].reshape(C, 1)
    in_maps = []
    for core in range(8):
        b, half = core // 2, core % 2
        q0 = half * NQ
        xrr = np.ascontiguousarray(np.roll(xr[b], -q0, axis=1))
        xir = np.ascontiguousarray(np.roll(xi[b], -q0, axis=1))
        in_maps.append({
            **common,
            "x16r": pack16(xrr), "x16i": pack16(xir),
            "xT8r": packT8(xrr), "xT8i": packT8(xir),
            "xr32": np.ascontiguousarray(xrr + pbr_col),
            "xi32": np.ascontiguousarray(xir + pbi_col),
        })

    nc = _build()
    trace = os.environ.get("BASS_KERNEL_TRACE") == "1"
    res = run_bass_kernel_spmd(nc, in_maps, core_ids=list(range(8)), trace=trace)
    kernel._last_result = res

    out = np.empty((B, C, N), np.complex64)
    for core in range(8):
        b, half = core // 2, core % 2
        q0 = half * NQ
        rr = res.results[core]
        out[b, :, q0:q0 + NQ] = rr["outr"] + 1j * rr["outi"]
    return out.reshape(B, C, HH, WW)


kernel._last_result = None
